# revision 1
# baseline (speedup 1.0000x reference)
"""Trainium2 Bass kernel for dilated local attention (nn_DilateAttention).

Problem: x [8, 64, 64, 256] f32, W_qkv [768, 256] f32.
  qkv = x @ W_qkv.T; per pixel, per head (8 heads x 32 dim): attention over
  the 9 dilated (3x3, dilation 3) spatial neighbors with zero padding.

Strategy: data-parallel over batch across 8 cores (1 image per core).
On-chip layout is transposed [c, m] (m = flat pixel index) so each of the
9 neighbor offsets delta = 64*dr + dc is a free-dim slice of a zero-border-
padded k/v buffer. PE does the qkv projection (float32r), the per-head
score reduction (bf16 product tile as stationary operand against a
block-ones moving operand, giving scores in [m, head*9+kk] layout), the
attn broadcast over head dims, and the weighted-sum accumulation (identity
lhsT, PSUM accumulate) in bf16. DVE does elementwise muls and the softmax
reductions, ACT does exp (with the 1/sqrt(dph) scale folded in) and PSUM
evacuations. Column-wrap reads are fixed with a 0/1 mask plus a count
correction on the softmax denominator (reference zero-pads keys, so invalid
slots contribute exp(0)=1 to the denominator and 0 to the numerator).
"""

import sys

sys.path.insert(0, "/opt/trn_rl_repo")

import numpy as np
import ml_dtypes
from contextlib import ExitStack

import concourse.bass as bass
import concourse.bacc as bacc
import concourse.tile as tile
from concourse import mybir
from concourse.bass_utils import run_bass_kernel_spmd

B, H, W, C = 8, 64, 64, 256
NH, DPH, K2 = 8, 32, 9
N = H * W          # 4096 pixels
PAD = 256          # zero border on each side of k/v (covers |delta| <= 195)
MCH = 512          # pixels per m-chunk
NCH = N // MCH     # 8 chunks
NSUB = N // 128    # 32 m-subchunks (scores/softmax granularity)
SCALE = DPH ** -0.5
F32 = mybir.dt.float32
F32R = mybir.dt.float32r
BF16 = mybir.dt.bfloat16
NPBF16 = ml_dtypes.bfloat16

DELTAS = [64 * (3 * i - 3) + (3 * j - 3) for i in range(3) for j in range(3)]


def _host_consts():
    ident = np.eye(128, dtype=np.float32)
    identb = np.eye(128, dtype=NPBF16)
    # score reduce (moving operand): ones_s[p, nn] = 1 iff p//32 == nn
    # (the 1/sqrt(dph) scale is applied inside the exp activation)
    ones_s = np.zeros((128, 4), NPBF16)
    for p in range(128):
        ones_s[p, p // 32] = 1.0
    # attn broadcast: B[p, j, kk, q] = 1 iff p == (4j + q//32)*9 + kk
    bkk = np.zeros((72, 2, 9, 128), NPBF16)
    for jj in range(2):
        for kk in range(9):
            for q in range(128):
                bkk[(4 * jj + q // 32) * 9 + kk, jj, kk, q] = 1.0
    bkk = bkk.reshape(72, 2 * 9 * 128)
    # r broadcast over kk within [72, m] layout: b9[h, p] = 1 iff p//9 == h
    b9 = np.zeros((8, 72), np.float32)
    for p in range(72):
        b9[p // 9, p] = 1.0
    # column-validity mask in [m-sub, f=h*9+kk] layout, plus invalid counts
    maskT = np.zeros((128, NSUB, 72), NPBF16)
    cntT = np.zeros((128, NSUB), np.float32)
    for ms in range(NSUB):
        m = ms * 128 + np.arange(128)
        jm = m % 64
        for kk in range(9):
            dc = 3 * (kk % 3) - 3
            valid = (((jm + dc) >= 0) & ((jm + dc) < 64)).astype(np.float32)
            for h in range(8):
                maskT[:, ms, h * 9 + kk] = valid
            cntT[:, ms] += 1.0 - valid
    maskT = maskT.reshape(128, NSUB * 72)
    return ident, identb, ones_s, bkk, b9, maskT, cntT


def build_nc() -> bass.Bass:
    nc = bacc.Bacc()
    x_d = nc.declare_dram_parameter("x", [N, C], F32, isOutput=False)
    w_d = nc.declare_dram_parameter("w", [3 * C, C], F32, isOutput=False)
    ident_d = nc.declare_dram_parameter("ident", [128, 128], F32, isOutput=False)
    identb_d = nc.declare_dram_parameter("identb", [128, 128], BF16, isOutput=False)
    ones_s_d = nc.declare_dram_parameter("ones_s", [128, 4], BF16, isOutput=False)
    bkk_d = nc.declare_dram_parameter("bkk", [72, 2 * 9 * 128], BF16, isOutput=False)
    b9_d = nc.declare_dram_parameter("b9", [8, 72], F32, isOutput=False)
    maskT_d = nc.declare_dram_parameter("maskT", [128, NSUB * 72], BF16, isOutput=False)
    cntT_d = nc.declare_dram_parameter("cntT", [128, NSUB], F32, isOutput=False)
    out_d = nc.declare_dram_parameter("out", [N, C], F32, isOutput=True)

    with tile.TileContext(nc) as tc, ExitStack() as ctx:
        # ---- persistent SBUF pools ----
        singles = ctx.enter_context(tc.tile_pool(name="singles", bufs=1))
        qkv_pool = ctx.enter_context(tc.tile_pool(name="qkv", bufs=1))

        ident = singles.tile([128, 128], F32)
        nc.gpsimd.dma_start(out=ident, in_=ident_d[:, :])
        identb = singles.tile([128, 128], BF16)
        nc.gpsimd.dma_start(out=identb, in_=identb_d[:, :])
        ones_s = singles.tile([128, 4], BF16)
        nc.gpsimd.dma_start(out=ones_s, in_=ones_s_d[:, :])
        bkk = singles.tile([72, 2, 9, 128], BF16)
        nc.gpsimd.dma_start(
            out=bkk, in_=bkk_d[:, :].rearrange("p (j k q) -> p j k q", j=2, k=9))
        b9t = singles.tile([8, 72], F32)
        nc.gpsimd.dma_start(out=b9t, in_=b9_d[:, :])
        maskT = singles.tile([128, NSUB, 72], BF16)
        nc.gpsimd.dma_start(
            out=maskT, in_=maskT_d[:, :].rearrange("p (s f) -> p s f", f=72))
        cntT = singles.tile([128, NSUB], F32)
        nc.gpsimd.dma_start(out=cntT, in_=cntT_d[:, :])

        # q/k/v in transposed [c, m] bf16 layout; k/v have zero borders of PAD
        qT = [qkv_pool.tile([128, N], BF16, name=f"qT{j}") for j in range(2)]
        kT = [qkv_pool.tile([128, N + 2 * PAD], BF16, name=f"kT{j}") for j in range(2)]
        vT = [qkv_pool.tile([128, N + 2 * PAD], BF16, name=f"vT{j}") for j in range(2)]
        for j in range(2):
            nc.gpsimd.memset(kT[j][:, 0:PAD], 0.0)
            nc.gpsimd.memset(kT[j][:, PAD + N:], 0.0)
            nc.gpsimd.memset(vT[j][:, 0:PAD], 0.0)
            nc.gpsimd.memset(vT[j][:, PAD + N:], 0.0)

        # ---- P0+P1: W^T tiles and x^T via PE transpose ----
        xt_pool = tc.alloc_tile_pool(name="xt_pool", bufs=1)
        with tc.tile_pool(name="trans_sb", bufs=4) as tsb, \
             tc.tile_pool(name="trans_ps", bufs=2, space="PSUM") as tps:
            wlhsT = [singles.tile([128, 6, 128], F32R, name=f"wlhsT{j}") for j in range(2)]
            for ot in range(6):
                w_rows = tsb.tile([128, 256], F32, name="w_rows")
                nc.gpsimd.dma_start(out=w_rows, in_=w_d[ot * 128:(ot + 1) * 128, :])
                for j in range(2):
                    wt_ps = tps.tile([128, 128], F32, name="wt_ps")
                    nc.tensor.transpose(wt_ps, w_rows[:, j * 128:(j + 1) * 128], ident)
                    nc.scalar.copy(out=wlhsT[j][:, ot, :], in_=wt_ps)

            xT = [xt_pool.tile([128, N], F32R, name=f"xT{j}") for j in range(2)]
            for mt in range(32):
                x_rows = tsb.tile([128, 256], F32, name="x_rows")
                nc.gpsimd.dma_start(out=x_rows, in_=x_d[mt * 128:(mt + 1) * 128, :])
                xt_ps = tps.tile([128, 256], F32, name="xt_ps")
                for j in range(2):
                    nc.tensor.transpose(
                        xt_ps[:, j * 128:(j + 1) * 128],
                        x_rows[:, j * 128:(j + 1) * 128], ident)
                for j in range(2):
                    nc.vector.tensor_copy(
                        out=xT[j][:, mt * 128:(mt + 1) * 128],
                        in_=xt_ps[:, j * 128:(j + 1) * 128])

        # ---- P2: qkv projection (f32r) -> bf16 qT/kT/vT ----
        with tc.tile_pool(name="qkv_ps", bufs=4, space="PSUM") as qps:
            for ot in range(6):
                for ch in range(NCH):
                    acc = qps.tile([128, MCH], F32, name="acc")
                    for j in range(2):
                        nc.tensor.matmul(
                            acc, wlhsT[j][:, ot, :],
                            xT[j][:, ch * MCH:(ch + 1) * MCH],
                            start=(j == 0), stop=(j == 1))
                    dst_j = ot % 2
                    if ot < 2:
                        dst = qT[dst_j][:, ch * MCH:(ch + 1) * MCH]
                    elif ot < 4:
                        dst = kT[dst_j][:, PAD + ch * MCH:PAD + (ch + 1) * MCH]
                    else:
                        dst = vT[dst_j][:, PAD + ch * MCH:PAD + (ch + 1) * MCH]
                    if ot % 2 == 0:
                        nc.scalar.copy(out=dst, in_=acc)
                    else:
                        nc.vector.tensor_copy(out=dst, in_=acc)
        xt_pool.release()

        # ---- P3: scores + softmax (m on partitions), then transpose back ----
        attn_pool = ctx.enter_context(tc.tile_pool(name="attn_sb", bufs=1))
        attn72 = attn_pool.tile([72, N], BF16)  # normalized attn weights
        r72 = attn_pool.tile([8, N], F32)       # per-head softmax reciprocal

        with tc.tile_pool(name="sc_sb", bufs=6) as ssb, \
             tc.tile_pool(name="sm_sb", bufs=8) as smb, \
             tc.tile_pool(name="st_ps", bufs=4, space="PSUM") as sps, \
             tc.tile_pool(name="at_ps", bufs=2, space="PSUM") as aps, \
             tc.tile_pool(name="rt_ps", bufs=1, space="PSUM") as rps, \
             tc.tile_pool(name="rb72_ps", bufs=1, space="PSUM") as rbps3:
            for ch in range(NCH):
                s_t = [sps.tile([128, 72], F32, name="s_t") for _ in range(4)]
                for kk in range(K2):
                    dl = DELTAS[kk]
                    for j in range(2):
                        t_t = ssb.tile([128, MCH], BF16, name="t_t")
                        nc.vector.tensor_mul(
                            t_t, qT[j][:, ch * MCH:(ch + 1) * MCH],
                            kT[j][:, PAD + ch * MCH + dl:PAD + (ch + 1) * MCH + dl])
                        for sub in range(4):
                            out_ap = s_t[sub].rearrange(
                                "p (h k) -> p h k", k=9)[:, 4 * j:4 * j + 4, kk]
                            nc.tensor.matmul(
                                out_ap, t_t[:, sub * 128:(sub + 1) * 128],
                                ones_s, start=True, stop=True)
                at_ps = aps.tile([72, 4, 128], BF16, name="at_ps")
                rt_ps = rps.tile([8, 4, 128], F32, name="rt_ps")
                for sub in range(4):
                    ms = ch * 4 + sub
                    e_t = smb.tile([128, 72], BF16, name="e_t")
                    nc.scalar.activation(
                        e_t, s_t[sub], mybir.ActivationFunctionType.Exp,
                        scale=float(SCALE))
                    em_t = smb.tile([128, 72], BF16, name="em_t")
                    nc.vector.tensor_mul(em_t, e_t, maskT[:, ms, :])
                    den = smb.tile([128, 8], F32, name="den")
                    nc.vector.reduce_sum(
                        den, em_t.rearrange("p (h k) -> p h k", k=9),
                        axis=mybir.AxisListType.X)
                    nc.vector.tensor_scalar_add(
                        out=den, in0=den, scalar1=cntT[:, ms:ms + 1])
                    rr = smb.tile([128, 8], F32, name="rr")
                    nc.vector.reciprocal(rr, den)
                    nc.tensor.transpose(at_ps[:, sub, :], em_t, identb)
                    nc.tensor.transpose(rt_ps[:, sub, :], rr, ident)
                nc.scalar.copy(
                    out=attn72[:, ch * MCH:(ch + 1) * MCH],
                    in_=at_ps.rearrange("p s q -> p (s q)"))
                nc.scalar.copy(
                    out=r72[:, ch * MCH:(ch + 1) * MCH],
                    in_=rt_ps.rearrange("p s q -> p (s q)"))
                rb72 = rbps3.tile([72, MCH], F32, name="rb72")
                nc.tensor.matmul(rb72, b9t, r72[:, ch * MCH:(ch + 1) * MCH],
                                 start=True, stop=True)
                nc.vector.tensor_mul(
                    attn72[:, ch * MCH:(ch + 1) * MCH],
                    attn72[:, ch * MCH:(ch + 1) * MCH], rb72)

        # ---- P4: weighted sum of v, transpose back, store ----
        with tc.tile_pool(name="av_sb", bufs=6) as asb, \
             tc.tile_pool(name="o_sb", bufs=4) as osb, \
             tc.tile_pool(name="bc_ps", bufs=3, space="PSUM") as bps, \
             tc.tile_pool(name="out_ps", bufs=2, space="PSUM") as ops, \
             tc.tile_pool(name="bt_ps", bufs=2, space="PSUM") as btps:
            out_view = out_d[:, :].rearrange(
                "(a t p) (j c) -> a p t j c", t=4, p=128, j=2)
            for ch in range(NCH):
                a_sl = attn72[:, ch * MCH:(ch + 1) * MCH]
                for j in range(2):
                    o_ps = ops.tile([128, MCH], F32, name="o_ps")
                    for kk in range(K2):
                        dl = DELTAS[kk]
                        bc_ps = bps.tile([128, MCH], F32, name="bc_ps")
                        nc.tensor.matmul(bc_ps, bkk[:, j, kk, :], a_sl,
                                         start=True, stop=True)
                        bc_sb = asb.tile([128, MCH], BF16, name="bc_sb")
                        nc.scalar.copy(out=bc_sb, in_=bc_ps)
                        t2 = asb.tile([128, MCH], BF16, name="t2")
                        nc.vector.tensor_mul(
                            t2, bc_sb,
                            vT[j][:, PAD + ch * MCH + dl:PAD + (ch + 1) * MCH + dl])
                        nc.tensor.matmul(o_ps, identb, t2,
                                         start=(kk == 0), stop=(kk == K2 - 1))
                    o_norm = osb.tile([128, MCH], F32, name="o_norm")
                    nc.vector.tensor_copy(out=o_norm, in_=o_ps)
                    bt_ps = btps.tile([128, 4, 128], F32, name="bt_ps")
                    for tt in range(4):
                        nc.tensor.transpose(
                            bt_ps[:, tt, :], o_norm[:, tt * 128:(tt + 1) * 128],
                            ident)
                    o_fin = osb.tile([128, 4, 128], F32, name="o_fin")
                    nc.scalar.copy(out=o_fin, in_=bt_ps)
                    nc.sync.dma_start(out=out_view[ch, :, :, j, :], in_=o_fin)
    nc.compile()
    return nc


_NC_CACHE = None


def kernel(x: np.ndarray, W_qkv: np.ndarray) -> np.ndarray:
    global _NC_CACHE
    if _NC_CACHE is None:
        _NC_CACHE = build_nc()
    nc = _NC_CACHE

    x = np.ascontiguousarray(x, dtype=np.float32)
    W_qkv = np.ascontiguousarray(W_qkv, dtype=np.float32)
    ident, identb, ones_s, bkk, b9, maskT, cntT = _host_consts()
    consts = {
        "w": W_qkv, "ident": ident, "identb": identb, "ones_s": ones_s,
        "bkk": bkk, "b9": b9, "maskT": maskT, "cntT": cntT,
    }
    in_maps = [
        {"x": x[b].reshape(N, C).copy(), **consts} for b in range(B)
    ]
    res = run_bass_kernel_spmd(nc, in_maps, list(range(B)))
    out = np.stack([res.results[b]["out"].reshape(H, W, C) for b in range(B)])
    return out


if __name__ == "__main__":
    rng = np.random.default_rng(0)
    x = rng.standard_normal((B, H, W, C), dtype=np.float32)
    wq = (rng.standard_normal((3 * C, C), dtype=np.float32) * 0.02).astype(np.float32)
    out = kernel(x, wq)
    print("out", out.shape, out.dtype, float(np.abs(out).mean()))



# revision 3
# speedup vs baseline: 1.1968x; 1.1968x over previous
"""Trainium2 Bass kernel v2 for dilated local attention (nn_DilateAttention).

Problem: x [8, 64, 64, 256] f32, W_qkv [768, 256] f32.
  qkv = x @ W_qkv.T; per pixel, per head (8 heads x 32 dim): attention over
  the 9 dilated (3x3, dilation 3) spatial neighbors with zero padding.

Strategy (data-parallel over batch, 1 image per core), [c, m] on-chip layout:
  - PE transposes x (f32r, 1.5 cyc/row), f32r projection, per-head score
    reduction with product-as-stationary matmuls (free=4 per (sub,kk,j)),
    column-wrap mask folded into PSUM scores via an identity matmul of a
    -LARGE mask (no DVE mask multiply), and AV accumulation via
    transpose-accumulate matmuls that produce rows-layout output directly.
  - DVE does the q*k / attn*v elementwise products (bf16 2x mode) and the
    softmax chain, batched 4 m-subs at a time ([128, 4, 72] PSUM score
    tiles, single bank).
  - Attention normalization (1/den) is applied in [m, 72] layout before
    transposing, with a free-broadcast scalar_tensor_tensor.
  - The attn broadcast over channel partitions runs on a per-unit routed
    mix of PE matmul (+Act/DVE/Pool PSUM evacuation) and SBUF->SBUF DMA
    with a stride-0 partition-replicating access pattern.
  - Output is stored from rows-layout SBUF by DMA, 8 m-subs per transfer.
"""

import sys

sys.path.insert(0, "/opt/trn_rl_repo")

import numpy as np
import ml_dtypes
from contextlib import ExitStack

import concourse.bass as bass
import concourse.bacc as bacc
import concourse.tile as tile
from concourse import mybir
from concourse.bass_utils import run_bass_kernel_spmd

B, H, W, C = 8, 64, 64, 256
NH, DPH, K2 = 8, 32, 9
N = H * W          # 4096 pixels
PAD = 256          # zero border on each side of k/v (covers |delta| <= 195)
MCH = 1024         # pixels per m-chunk in P2/P3
NCH = N // MCH     # 4 chunks
SUBS = MCH // 128  # 8 m-subs per chunk
NSUB = N // 128    # 32 m-subs
SCALE = DPH ** -0.5
MASKNEG = -60.0 / SCALE   # exp(scale*(s + MASKNEG)) = exp(scale*s - 60) ~ 0
F32 = mybir.dt.float32
F32R = mybir.dt.float32r
BF16 = mybir.dt.bfloat16
NPBF16 = ml_dtypes.bfloat16

DELTAS = [64 * (3 * i - 3) + (3 * j - 3) for i in range(3) for j in range(3)]

# broadcast route per (ch, j, kk) unit: 'pe_act', 'pe_dve', 'pe_pool', 'dma'
# 72 units total; tuned against the CoreSim profile.
BC_ROUTES = {}
for _ch in range(NCH):
    for _j in range(2):
        for _kk in range(K2):
            r = ['dma', 'pe_act', 'dma', 'pe_dve', 'dma', 'pe_act',
                 'dma', 'pe_dve', 'pe_act'][_kk]
            BC_ROUTES[(_ch, _j, _kk)] = r

# product engine per (phase, ch, j, kk): 'dve' or 'pool'
def _prod_engine(phase, ch, j, kk):
    # Pool muls cost ~0.833/elem in-model; give it a third of the products
    if kk in (1, 4, 7):
        return 'pool'
    return 'dve'


def _host_consts():
    ident = np.eye(128, dtype=np.float32)
    identb = np.eye(128, dtype=NPBF16)
    # score reduce (moving operand): ones_s[p, hh] = 1 iff p//32 == hh
    ones_s = np.zeros((128, 4), NPBF16)
    for p in range(128):
        ones_s[p, p // 32] = 1.0
    # bc matmul stationary: bkk[p, q] for (j, kk): p = attn72 row index
    # (h*9+kk), q = channel 0..127 -> 1 iff p == (4j + q//32)*9 + kk
    bkk = np.zeros((72, 2, 9, 128), NPBF16)
    for jj in range(2):
        for kk in range(9):
            for q in range(128):
                bkk[(4 * jj + q // 32) * 9 + kk, jj, kk, q] = 1.0
    bkk = bkk.reshape(72, 2 * 9 * 128)
    # column-validity 0/1 mask in [m-sub, f=h*9+kk] layout, plus invalid
    # counts for the denominator (reference zero-pads keys: invalid slots
    # contribute exp(0)=1 to the denominator and 0 to the numerator).
    maskA = np.zeros((128, NSUB, 72), NPBF16)
    cntT = np.zeros((128, NSUB), np.float32)
    for ms in range(NSUB):
        m = ms * 128 + np.arange(128)
        jm = m % 64
        for kk in range(9):
            dc = 3 * (kk % 3) - 3
            valid = (((jm + dc) >= 0) & ((jm + dc) < 64)).astype(np.float32)
            for h in range(8):
                maskA[:, ms, h * 9 + kk] = valid
            cntT[:, ms] += 1.0 - valid
    maskA = maskA.reshape(128, NSUB * 72)
    return ident, identb, ones_s, bkk, maskA, cntT


def build_nc() -> bass.Bass:
    nc = bacc.Bacc()
    x_d = nc.declare_dram_parameter("x", [N, C], F32, isOutput=False)
    w_d = nc.declare_dram_parameter("w", [3 * C, C], F32, isOutput=False)
    ident_d = nc.declare_dram_parameter("ident", [128, 128], F32, isOutput=False)
    identb_d = nc.declare_dram_parameter("identb", [128, 128], BF16, isOutput=False)
    ones_s_d = nc.declare_dram_parameter("ones_s", [128, 4], BF16, isOutput=False)
    bkk_d = nc.declare_dram_parameter("bkk", [72, 2 * 9 * 128], BF16, isOutput=False)
    maskA_d = nc.declare_dram_parameter("maskA", [128, NSUB * 72], BF16, isOutput=False)
    cntT_d = nc.declare_dram_parameter("cntT", [128, NSUB], F32, isOutput=False)
    out_d = nc.declare_dram_parameter("out", [N, C], F32, isOutput=True)
    at_d = nc.dram_tensor("at_scratch", [80, N], BF16, kind="Internal")[:, :]

    with tile.TileContext(nc) as tc, ExitStack() as ctx:
        # ---- persistent SBUF pools ----
        singles = ctx.enter_context(tc.tile_pool(name="singles", bufs=1))
        qkv_pool = ctx.enter_context(tc.tile_pool(name="qkv", bufs=1))

        ident = singles.tile([128, 128], F32)
        nc.gpsimd.dma_start(out=ident, in_=ident_d[:, :])
        identr = singles.tile([128, 128], F32R)
        nc.gpsimd.dma_start(out=identr, in_=ident_d[:, :])
        identb = singles.tile([128, 128], BF16)
        nc.gpsimd.dma_start(out=identb, in_=identb_d[:, :])
        ones_s = singles.tile([128, 4], BF16)
        nc.gpsimd.dma_start(out=ones_s, in_=ones_s_d[:, :])
        bkk = singles.tile([72, 2, 9, 128], BF16)
        nc.gpsimd.dma_start(
            out=bkk, in_=bkk_d[:, :].rearrange("p (j k q) -> p j k q", j=2, k=9))
        maskA = singles.tile([128, NSUB, 72], BF16)
        nc.gpsimd.dma_start(
            out=maskA, in_=maskA_d[:, :].rearrange("p (s f) -> p s f", f=72))
        cntT = singles.tile([128, NSUB], F32)
        nc.gpsimd.dma_start(out=cntT, in_=cntT_d[:, :])

        # q/k/v in transposed [c, m] bf16 layout; k/v have zero borders of PAD
        qT = [qkv_pool.tile([128, N], BF16, name=f"qT{j}") for j in range(2)]
        kT = [qkv_pool.tile([128, N + 2 * PAD], BF16, name=f"kT{j}") for j in range(2)]
        vT = [qkv_pool.tile([128, N + 2 * PAD], BF16, name=f"vT{j}") for j in range(2)]
        for j in range(2):
            nc.gpsimd.memset(kT[j][:, 0:PAD], 0.0)
            nc.gpsimd.memset(kT[j][:, PAD + N:], 0.0)
            nc.gpsimd.memset(vT[j][:, 0:PAD], 0.0)
            nc.gpsimd.memset(vT[j][:, PAD + N:], 0.0)

        # normalized attention, [72 rows = h*9+kk, m]
        attn_pool = ctx.enter_context(tc.tile_pool(name="attn_sb", bufs=1))
        at72 = attn_pool.tile([72, N], BF16)

        # ---- P0+P1: W^T tiles and x^T via PE transpose (f32r) ----
        xt_pool = tc.alloc_tile_pool(name="xt_pool", bufs=1)
        with tc.tile_pool(name="trans_sb", bufs=4) as tsb, \
             tc.tile_pool(name="trans_ps", bufs=2, space="PSUM") as tps:
            wlhsT = [singles.tile([128, 6, 128], F32R, name=f"wlhsT{j}") for j in range(2)]
            for ot in range(6):
                w_rows = tsb.tile([128, 256], F32R, name="w_rows")
                nc.gpsimd.dma_start(out=w_rows, in_=w_d[ot * 128:(ot + 1) * 128, :])
                for j in range(2):
                    wt_ps = tps.tile([128, 128], F32R, name="wt_ps")
                    nc.tensor.transpose(wt_ps, w_rows[:, j * 128:(j + 1) * 128], identr)
                    nc.scalar.copy(out=wlhsT[j][:, ot, :], in_=wt_ps)

            xT = [xt_pool.tile([128, N], F32R, name=f"xT{j}") for j in range(2)]
            xin = x_d[:, :].rearrange("(t p) c -> p t c", p=128).bitcast(F32R)
            for mb in range(8):
                x_rows = tsb.tile([128, 4, 256], F32R, name="x_rows")
                qeng = [nc.sync, nc.gpsimd, nc.scalar][mb % 3]
                qeng.dma_start(out=x_rows, in_=xin[:, mb * 4:(mb + 1) * 4, :])
                for t in range(4):
                    mt = mb * 4 + t
                    xt_ps = tps.tile([128, 256], F32R, name="xt_ps")
                    for j in range(2):
                        nc.tensor.transpose(
                            xt_ps[:, j * 128:(j + 1) * 128],
                            x_rows[:, t, j * 128:(j + 1) * 128], identr)
                    for j in range(2):
                        r = (mt * 2 + j) % 2
                        dst = xT[j][:, mt * 128:(mt + 1) * 128]
                        src = xt_ps[:, j * 128:(j + 1) * 128]
                        if r == 0:
                            nc.vector.tensor_copy(out=dst, in_=src)
                        else:
                            nc.scalar.copy(out=dst, in_=src)

        # ---- P2: qkv projection (f32r) -> bf16 qT/kT/vT ----
        with tc.tile_pool(name="qkv_ps", bufs=4, space="PSUM") as qps:
            for ot in range(6):
                for ch in range(8):
                    acc = qps.tile([128, 512], F32, name="acc")
                    for j in range(2):
                        nc.tensor.matmul(
                            acc, wlhsT[j][:, ot, :],
                            xT[j][:, ch * 512:(ch + 1) * 512],
                            start=(j == 0), stop=(j == 1))
                    dst_j = ot % 2
                    if ot < 2:
                        dst = qT[dst_j][:, ch * 512:(ch + 1) * 512]
                    elif ot < 4:
                        dst = kT[dst_j][:, PAD + ch * 512:PAD + (ch + 1) * 512]
                    else:
                        dst = vT[dst_j][:, PAD + ch * 512:PAD + (ch + 1) * 512]
                    r = (ot * 8 + ch) % 2
                    if r == 0:
                        nc.scalar.copy(out=dst, in_=acc)
                    else:
                        nc.vector.tensor_copy(out=dst, in_=acc)
        xt_pool.release()

        # ---- P3 + P4 per m-chunk: scores, softmax, attn transpose, AV ----
        out_view = out_d[:, :].rearrange(
            "(s p) (j c) -> p s j c", p=128, j=2)
        with tc.tile_pool(name="prod_sb", bufs=2) as ttb, \
             tc.tile_pool(name="sc_ps", bufs=1, space="PSUM") as sps, \
             tc.tile_pool(name="sm_sb", bufs=4) as smb, \
             tc.tile_pool(name="at_ps", bufs=1, space="PSUM") as aps, \
             tc.tile_pool(name="bc_ps", bufs=1, space="PSUM") as bps, \
             tc.tile_pool(name="bc_sb", bufs=1) as bsb, \
             tc.tile_pool(name="o_ps", bufs=1, space="PSUM") as ops, \
             tc.tile_pool(name="o_sb", bufs=2) as osb:
            for ch in range(NCH):
                m0 = ch * MCH
                # -- products q*k_delta for this chunk --
                t_t = [[None] * K2 for _ in range(2)]
                for kk in range(K2):
                    dl = DELTAS[kk]
                    for j in range(2):
                        t = ttb.tile([128, MCH], BF16, name=f"pr{j}_{kk}")
                        eng = _prod_engine('qk', ch, j, kk)
                        if eng == 'pool':
                            nc.gpsimd.tensor_mul(
                                t, qT[j][:, m0:m0 + MCH],
                                kT[j][:, PAD + m0 + dl:PAD + m0 + MCH + dl])
                        else:
                            nc.vector.tensor_mul(
                                t, qT[j][:, m0:m0 + MCH],
                                kT[j][:, PAD + m0 + dl:PAD + m0 + MCH + dl])
                        t_t[j][kk] = t

                # -- scores + softmax, 4 m-subs at a time --
                for g in range(SUBS // 4):
                    s_ps = sps.tile([128, 4, 72], F32, name="s_ps")
                    for sub4 in range(4):
                        sub = g * 4 + sub4
                        for kk in range(K2):
                            for j in range(2):
                                out_ap = s_ps.rearrange(
                                    "p s (h k) -> p s h k", k=9)[:, sub4, 4 * j:4 * j + 4, kk]
                                nc.tensor.matmul(
                                    out_ap,
                                    t_t[j][kk][:, sub * 128:sub * 128 + 128],
                                    ones_s, start=True, stop=True)
                    ms0 = ch * SUBS + g * 4
                    em0 = smb.tile([128, 4, 72], BF16, name="em0")
                    nc.scalar.activation(
                        em0, s_ps, mybir.ActivationFunctionType.Exp,
                        scale=float(SCALE))
                    em = smb.tile([128, 4, 72], BF16, name="em")
                    nc.vector.tensor_mul(em, em0, maskA[:, ms0:ms0 + 4, :])
                    den = smb.tile([128, 4, 8], F32, name="den")
                    nc.vector.reduce_sum(
                        den, em.rearrange("p s (h k) -> p s h k", k=9),
                        axis=mybir.AxisListType.X)
                    cb = cntT[:, ms0:ms0 + 4].unsqueeze(2).broadcast_to([128, 4, 8])
                    nc.vector.scalar_tensor_tensor(
                        out=den, in0=den, scalar=1.0, in1=cb,
                        op0=mybir.AluOpType.mult, op1=mybir.AluOpType.add)
                    rr = smb.tile([128, 4, 8], F32, name="rr")
                    nc.vector.reciprocal(rr, den)
                    rrb = smb.tile([128, 4, 8], BF16, name="rrb")
                    nc.vector.tensor_copy(out=rrb, in_=rr)
                    # normalize in [m, 72] with free-broadcast of rrb over kk
                    emn = smb.tile([128, 4, 72], BF16, name="emn")
                    rbc = rrb[:, :, :].unsqueeze(3).broadcast_to([128, 4, 8, 9])
                    nc.vector.scalar_tensor_tensor(
                        out=emn.rearrange("p s (h k) -> p s h k", k=9),
                        in0=em.rearrange("p s (h k) -> p s h k", k=9),
                        scalar=1.0, in1=rbc,
                        op0=mybir.AluOpType.mult, op1=mybir.AluOpType.mult)
                    # transpose to [72, m]
                    at_ps = aps.tile([72, 4, 128], F32, name="at_ps")
                    for sub4 in range(4):
                        nc.tensor.matmul(
                            at_ps[:, sub4, :], emn[:, sub4, :], identb,
                            start=True, stop=True)
                    nc.scalar.copy(
                        out=at72[:, m0 + g * 512:m0 + (g + 1) * 512],
                        in_=at_ps.rearrange("p s q -> p (s q)"))

                nc.sync.dma_start(out=at_d[0:72, m0:m0 + MCH],
                                  in_=at72[:, m0:m0 + MCH])
                # -- AV: prefetch dma broadcasts, then bc/product/accumulate --
                bc_t = [[None] * K2 for _ in range(2)]
                for j in range(2):
                    for kk in range(K2):
                        if BC_ROUTES[(ch, j, kk)] != 'dma':
                            continue
                        bc = bsb.tile([128, MCH], BF16, name=f"bcd{j}_{kk}")
                        r0 = (4 * j) * 9 + kk
                        bap = at_d[r0:r0 + 28:9, m0:m0 + MCH]
                        bap = bap.unsqueeze(1).broadcast_to([4, 32, MCH])
                        qeng = [nc.sync, nc.gpsimd][(j + kk) % 2]
                        qeng.dma_start(out=bc, in_=bap)
                        bc_t[j][kk] = bc
                o_sb = osb.tile([128, SUBS, 256], F32, name="o_sb")
                for half in range(2):
                    h0 = half * 512
                    o_gs = [ops.tile([128, 512], F32, name=f"o_g{sub4}")
                            for sub4 in range(4)]
                    for j in range(2):
                        for kk in range(K2):
                            dl = DELTAS[kk]
                            route = BC_ROUTES[(ch, j, kk)]
                            if route == 'dma':
                                bc = bc_t[j][kk]
                                bch = bc[:, h0:h0 + 512]
                            elif half == 0:
                                bc = bsb.tile([128, MCH], BF16, name=f"bc{kk % 3}")
                                bc_t[j][kk] = bc
                                bc_ps = bps.tile([128, MCH], F32, name="bc_ps")
                                for q in range(MCH // 512):
                                    nc.tensor.matmul(
                                        bc_ps[:, q * 512:(q + 1) * 512],
                                        bkk[:, j, kk, :],
                                        at72[:, m0 + q * 512:m0 + (q + 1) * 512],
                                        start=True, stop=True)
                                if route == 'pe_act':
                                    nc.scalar.copy(out=bc, in_=bc_ps)
                                else:
                                    nc.vector.tensor_copy(out=bc, in_=bc_ps)
                                bch = bc[:, h0:h0 + 512]
                            else:
                                bch = bc_t[j][kk][:, h0:h0 + 512]
                            t2 = ttb.tile([128, 512], BF16, name=f"pr{j}_{kk}")
                            eng = _prod_engine('av', ch, j, kk)
                            vsl = vT[j][:, PAD + m0 + h0 + dl:PAD + m0 + h0 + 512 + dl]
                            if eng == 'pool':
                                nc.gpsimd.tensor_mul(t2, bch, vsl)
                            else:
                                nc.vector.tensor_mul(t2, bch, vsl)
                            for sub4 in range(4):
                                nc.tensor.matmul(
                                    o_gs[sub4][:, j * 128:(j + 1) * 128],
                                    t2[:, sub4 * 128:(sub4 + 1) * 128], identb,
                                    start=(kk == 0), stop=(kk == K2 - 1))
                    for sub4 in range(4):
                        sub = half * 4 + sub4
                        for j in range(2):
                            r = (sub * 2 + j) % 2
                            dst = o_sb[:, sub, j * 128:(j + 1) * 128]
                            src = o_gs[sub4][:, j * 128:(j + 1) * 128]
                            if r == 0:
                                nc.scalar.copy(out=dst, in_=src)
                            else:
                                nc.vector.tensor_copy(out=dst, in_=src)
                nc.sync.dma_start(
                    out=out_view[:, ch * SUBS:(ch + 1) * SUBS, :, :],
                    in_=o_sb.rearrange("p s (j c) -> p s j c", j=2))
    nc.compile()
    return nc


_NC_CACHE = None


def kernel(x: np.ndarray, W_qkv: np.ndarray) -> np.ndarray:
    global _NC_CACHE
    if _NC_CACHE is None:
        _NC_CACHE = build_nc()
    nc = _NC_CACHE

    x = np.ascontiguousarray(x, dtype=np.float32)
    W_qkv = np.ascontiguousarray(W_qkv, dtype=np.float32)
    ident, identb, ones_s, bkk, maskA, cntT = _host_consts()
    consts = {
        "w": W_qkv, "ident": ident, "identb": identb, "ones_s": ones_s,
        "bkk": bkk, "maskA": maskA, "cntT": cntT,
    }
    in_maps = [
        {"x": x[b].reshape(N, C).copy(), **consts} for b in range(B)
    ]
    res = run_bass_kernel_spmd(nc, in_maps, list(range(B)))
    out = np.stack([res.results[b]["out"].reshape(H, W, C) for b in range(B)])
    return out


if __name__ == "__main__":
    rng = np.random.default_rng(0)
    x = rng.standard_normal((B, H, W, C), dtype=np.float32)
    wq = (rng.standard_normal((3 * C, C), dtype=np.float32) * 0.02).astype(np.float32)
    out = kernel(x, wq)
    print("out", out.shape, out.dtype, float(np.abs(out).mean()))


# revision 4
# speedup vs baseline: 1.6623x; 1.3891x over previous
"""Trainium2 Bass kernel v2 for dilated local attention (nn_DilateAttention).

Problem: x [8, 64, 64, 256] f32, W_qkv [768, 256] f32.
  qkv = x @ W_qkv.T; per pixel, per head (8 heads x 32 dim): attention over
  the 9 dilated (3x3, dilation 3) spatial neighbors with zero padding.

Strategy (data-parallel over batch, 1 image per core), [c, m] on-chip layout:
  - PE: f32r transposes of x/W, f32r qkv projection, per-head score
    reduction with product-as-stationary matmuls, and AV accumulation via
    transpose-accumulate matmuls producing rows-layout output directly.
  - DVE/Pool: the q*k / attn*v elementwise products (bf16, SBUF-only so
    Pool is legal); DVE also runs the softmax chain batched 4 m-subs at a
    time on a single-bank [128, 4, 72] PSUM score tile.
  - Attention normalization (1/den) is applied in [m, 72] layout before
    transposing (free-broadcast scalar_tensor_tensor), then the normalized
    attention is transposed to [72, m] and round-tripped through a DRAM
    scratch so the per-channel broadcast becomes a partition-replicating
    (stride-0) DMA read - no PE/PSUM broadcast or evacuation needed.
  - Emission is software-pipelined: chunk ch+1's products/scores/softmax
    are emitted before chunk ch's AV phase so the in-order engine queues
    interleave the two chunks.
"""

import sys

sys.path.insert(0, "/opt/trn_rl_repo")

import numpy as np
import ml_dtypes
from contextlib import ExitStack

import concourse.bass as bass
import concourse.bacc as bacc
import concourse.tile as tile
from concourse import mybir
from concourse.bass_utils import run_bass_kernel_spmd

B, H, W, C = 8, 64, 64, 256
NH, DPH, K2 = 8, 32, 9
N = H * W          # 4096 pixels
PAD = 256          # zero border on each side of k/v (covers |delta| <= 195)
MCH = 1024         # pixels per m-chunk
NCH = N // MCH     # 4 chunks
SUBS = MCH // 128  # 8 m-subs per chunk
NSUB = N // 128    # 32 m-subs
SCALE = DPH ** -0.5
F32 = mybir.dt.float32
F32R = mybir.dt.float32r
BF16 = mybir.dt.bfloat16
NPBF16 = ml_dtypes.bfloat16

DELTAS = [64 * (3 * i - 3) + (3 * j - 3) for i in range(3) for j in range(3)]


def _prod_engine(phase, ch, j, kk):
    # Pool products are SBUF-only (HW-legal) and cheap in-model.
    if kk in (1, 4, 7):
        return 'pool'
    return 'dve'


def _host_consts():
    ident = np.eye(128, dtype=np.float32)
    identb = np.eye(128, dtype=NPBF16)
    # score reduce (moving operand): ones_s[p, hh] = 1 iff p//32 == hh
    ones_s = np.zeros((128, 4), NPBF16)
    for p in range(128):
        ones_s[p, p // 32] = 1.0
    # kept for interface compat (unused when broadcast is DMA-only)
    bkk = np.zeros((72, 2, 9, 128), NPBF16)
    for jj in range(2):
        for kk in range(9):
            for q in range(128):
                bkk[(4 * jj + q // 32) * 9 + kk, jj, kk, q] = 1.0
    bkk = bkk.reshape(72, 2 * 9 * 128)
    # column-validity 0/1 mask in [m-sub, f=h*9+kk] layout, plus invalid
    # counts for the denominator (reference zero-pads keys: invalid slots
    # contribute exp(0)=1 to the denominator and 0 to the numerator).
    maskA = np.zeros((128, NSUB, 72), NPBF16)
    cntT = np.zeros((128, NSUB), np.float32)
    for ms in range(NSUB):
        m = ms * 128 + np.arange(128)
        jm = m % 64
        for kk in range(9):
            dc = 3 * (kk % 3) - 3
            valid = (((jm + dc) >= 0) & ((jm + dc) < 64)).astype(np.float32)
            for h in range(8):
                maskA[:, ms, h * 9 + kk] = valid
            cntT[:, ms] += 1.0 - valid
    maskA = maskA.reshape(128, NSUB * 72)
    return ident, identb, ones_s, bkk, maskA, cntT


def build_nc() -> bass.Bass:
    nc = bacc.Bacc()
    x_d = nc.declare_dram_parameter("x", [N, C], F32, isOutput=False)
    w_d = nc.declare_dram_parameter("w", [3 * C, C], F32, isOutput=False)
    ident_d = nc.declare_dram_parameter("ident", [128, 128], F32, isOutput=False)
    identb_d = nc.declare_dram_parameter("identb", [128, 128], BF16, isOutput=False)
    ones_s_d = nc.declare_dram_parameter("ones_s", [128, 4], BF16, isOutput=False)
    bkk_d = nc.declare_dram_parameter("bkk", [72, 2 * 9 * 128], BF16, isOutput=False)
    maskA_d = nc.declare_dram_parameter("maskA", [128, NSUB * 72], BF16, isOutput=False)
    cntT_d = nc.declare_dram_parameter("cntT", [128, NSUB], F32, isOutput=False)
    out_d = nc.declare_dram_parameter("out", [N, C], F32, isOutput=True)
    at_d = nc.dram_tensor("at_scratch", [80, N], BF16, kind="Internal")[:, :]

    with tile.TileContext(nc) as tc, ExitStack() as ctx:
        singles = ctx.enter_context(tc.tile_pool(name="singles", bufs=1))
        qkv_pool = ctx.enter_context(tc.tile_pool(name="qkv", bufs=1))

        identr = singles.tile([128, 128], F32R)
        nc.gpsimd.dma_start(out=identr, in_=ident_d[:, :])
        identb = singles.tile([128, 128], BF16)
        nc.gpsimd.dma_start(out=identb, in_=identb_d[:, :])
        ones_s = singles.tile([128, 4], BF16)
        nc.gpsimd.dma_start(out=ones_s, in_=ones_s_d[:, :])
        maskA = singles.tile([128, NSUB, 72], BF16)
        nc.gpsimd.dma_start(
            out=maskA, in_=maskA_d[:, :].rearrange("p (s f) -> p s f", f=72))
        cntT = singles.tile([128, NSUB], F32)
        nc.gpsimd.dma_start(out=cntT, in_=cntT_d[:, :])

        # q/k/v in transposed [c, m] bf16 layout; k/v have zero borders of PAD
        qT = [qkv_pool.tile([128, N], BF16, name=f"qT{j}") for j in range(2)]
        kT = [qkv_pool.tile([128, N + 2 * PAD], BF16, name=f"kT{j}") for j in range(2)]
        vT = [qkv_pool.tile([128, N + 2 * PAD], BF16, name=f"vT{j}") for j in range(2)]
        for j in range(2):
            nc.gpsimd.memset(kT[j][:, 0:PAD], 0.0)
            nc.gpsimd.memset(kT[j][:, PAD + N:], 0.0)
            nc.gpsimd.memset(vT[j][:, 0:PAD], 0.0)
            nc.gpsimd.memset(vT[j][:, PAD + N:], 0.0)

        # normalized attention, [72 rows = h*9+kk, m]
        attn_pool = ctx.enter_context(tc.tile_pool(name="attn_sb", bufs=1))
        at72 = attn_pool.tile([72, N], BF16)

        # ---- P1: W^T tiles and x^T via PE transpose (f32r) ----
        xt_pool = tc.alloc_tile_pool(name="xt_pool", bufs=1)
        with tc.tile_pool(name="trans_sb", bufs=4) as tsb, \
             tc.tile_pool(name="trans_ps", bufs=2, space="PSUM") as tps:
            wlhsT = [singles.tile([128, 6, 128], F32R, name=f"wlhsT{j}") for j in range(2)]
            for ot in range(6):
                w_rows = tsb.tile([128, 256], F32R, name="w_rows")
                nc.gpsimd.dma_start(out=w_rows, in_=w_d[ot * 128:(ot + 1) * 128, :])
                for j in range(2):
                    wt_ps = tps.tile([128, 128], F32R, name="wt_ps")
                    nc.tensor.transpose(wt_ps, w_rows[:, j * 128:(j + 1) * 128], identr)
                    nc.scalar.copy(out=wlhsT[j][:, ot, :], in_=wt_ps)

            xT = [xt_pool.tile([128, N], F32R, name=f"xT{j}") for j in range(2)]
            xin = x_d[:, :].rearrange("(t p) c -> p t c", p=128).bitcast(F32R)
            for mb in range(8):
                x_rows = tsb.tile([128, 4, 256], F32R, name="x_rows")
                qeng = [nc.sync, nc.gpsimd, nc.scalar][mb % 3]
                qeng.dma_start(out=x_rows, in_=xin[:, mb * 4:(mb + 1) * 4, :])
                for t in range(4):
                    mt = mb * 4 + t
                    xt_ps = tps.tile([128, 256], F32R, name="xt_ps")
                    for j in range(2):
                        nc.tensor.transpose(
                            xt_ps[:, j * 128:(j + 1) * 128],
                            x_rows[:, t, j * 128:(j + 1) * 128], identr)
                    for j in range(2):
                        dst = xT[j][:, mt * 128:(mt + 1) * 128]
                        src = xt_ps[:, j * 128:(j + 1) * 128]
                        if (mt * 2 + j) % 4 != 3:
                            nc.vector.tensor_copy(out=dst, in_=src)
                        else:
                            nc.scalar.copy(out=dst, in_=src)

        # ---- P2: qkv projection (f32r) -> bf16 qT/kT/vT ----
        with tc.tile_pool(name="qkv_ps", bufs=4, space="PSUM") as qps:
            for ot in range(6):
                for ch in range(8):
                    acc = qps.tile([128, 512], F32, name="acc")
                    for j in range(2):
                        nc.tensor.matmul(
                            acc, wlhsT[j][:, ot, :],
                            xT[j][:, ch * 512:(ch + 1) * 512],
                            start=(j == 0), stop=(j == 1))
                    dst_j = ot % 2
                    if ot < 2:
                        dst = qT[dst_j][:, ch * 512:(ch + 1) * 512]
                    elif ot < 4:
                        dst = kT[dst_j][:, PAD + ch * 512:PAD + (ch + 1) * 512]
                    else:
                        dst = vT[dst_j][:, PAD + ch * 512:PAD + (ch + 1) * 512]
                    if (ot * 8 + ch) % 3 == 0:
                        nc.scalar.copy(out=dst, in_=acc)
                    else:
                        nc.vector.tensor_copy(out=dst, in_=acc)
        xt_pool.release()

        # ---- P3/P4, software-pipelined over m-chunks ----
        out_view = out_d[:, :].rearrange(
            "(s p) (j c) -> p s j c", p=128, j=2)

        with tc.tile_pool(name="prod_sb", bufs=1) as ttb, \
             tc.tile_pool(name="t2_sb", bufs=2) as t2b, \
             tc.tile_pool(name="bcd_sb", bufs=1) as bsb, \
             tc.tile_pool(name="sm_sb", bufs=4) as smb, \
             tc.tile_pool(name="sc_ps", bufs=2, space="PSUM") as sps, \
             tc.tile_pool(name="at_ps", bufs=2, space="PSUM") as aps, \
             tc.tile_pool(name="o_ps", bufs=1, space="PSUM") as ops, \
             tc.tile_pool(name="o_sb", bufs=2) as osb:

            def front(ch):
                """products -> scores -> softmax -> normalized at72 -> DRAM."""
                m0 = ch * MCH
                t_t = [[None] * K2 for _ in range(2)]
                for kk in range(K2):
                    dl = DELTAS[kk]
                    for j in range(2):
                        t = ttb.tile([128, MCH], BF16, name=f"pr{j}_{kk}")
                        if _prod_engine('qk', ch, j, kk) == 'pool':
                            nc.gpsimd.tensor_mul(
                                t, qT[j][:, m0:m0 + MCH],
                                kT[j][:, PAD + m0 + dl:PAD + m0 + MCH + dl])
                        else:
                            nc.vector.tensor_mul(
                                t, qT[j][:, m0:m0 + MCH],
                                kT[j][:, PAD + m0 + dl:PAD + m0 + MCH + dl])
                        t_t[j][kk] = t
                for g in range(SUBS // 4):
                    s_ps = sps.tile([128, 4, 72], F32, name="s_ps")
                    for sub4 in range(4):
                        sub = g * 4 + sub4
                        for kk in range(K2):
                            for j in range(2):
                                out_ap = s_ps.rearrange(
                                    "p s (h k) -> p s h k", k=9)[:, sub4, 4 * j:4 * j + 4, kk]
                                nc.tensor.matmul(
                                    out_ap,
                                    t_t[j][kk][:, sub * 128:sub * 128 + 128],
                                    ones_s, start=True, stop=True)
                    ms0 = ch * SUBS + g * 4
                    em0 = smb.tile([128, 4, 72], BF16, name="em0")
                    nc.scalar.activation(
                        em0, s_ps, mybir.ActivationFunctionType.Exp,
                        scale=float(SCALE))
                    em = smb.tile([128, 4, 72], BF16, name="em")
                    nc.vector.tensor_mul(em, em0, maskA[:, ms0:ms0 + 4, :])
                    den = smb.tile([128, 4, 8], F32, name="den")
                    nc.vector.reduce_sum(
                        den, em.rearrange("p s (h k) -> p s h k", k=9),
                        axis=mybir.AxisListType.X)
                    cb = cntT[:, ms0:ms0 + 4].unsqueeze(2).broadcast_to([128, 4, 8])
                    nc.vector.scalar_tensor_tensor(
                        out=den, in0=den, scalar=1.0, in1=cb,
                        op0=mybir.AluOpType.mult, op1=mybir.AluOpType.add)
                    rr = smb.tile([128, 4, 8], F32, name="rr")
                    nc.vector.reciprocal(rr, den)
                    rrb = smb.tile([128, 4, 8], BF16, name="rrb")
                    nc.vector.tensor_copy(out=rrb, in_=rr)
                    emn = smb.tile([128, 4, 72], BF16, name="emn")
                    rbc = rrb[:, :, :].unsqueeze(3).broadcast_to([128, 4, 8, 9])
                    nc.vector.scalar_tensor_tensor(
                        out=emn.rearrange("p s (h k) -> p s h k", k=9),
                        in0=em.rearrange("p s (h k) -> p s h k", k=9),
                        scalar=1.0, in1=rbc,
                        op0=mybir.AluOpType.mult, op1=mybir.AluOpType.mult)
                    at_ps = aps.tile([72, 4, 128], F32, name="at_ps")
                    for sub4 in range(4):
                        nc.tensor.matmul(
                            at_ps[:, sub4, :], emn[:, sub4, :], identb,
                            start=True, stop=True)
                    nc.scalar.copy(
                        out=at72[:, m0 + g * 512:m0 + (g + 1) * 512],
                        in_=at_ps.rearrange("p s q -> p (s q)"))
                nc.sync.dma_start(out=at_d[0:72, m0:m0 + MCH],
                                  in_=at72[:, m0:m0 + MCH])

            def back(ch):
                """DMA-broadcast attn, attn*v products, transpose-accumulate."""
                m0 = ch * MCH
                bc_t = [[None] * K2 for _ in range(2)]
                nd = 0
                for kk in range(K2):
                    for j in range(2):
                        bc = bsb.tile([128, MCH], BF16, name=f"bcd{j}_{kk}")
                        r0 = (4 * j) * 9 + kk
                        bap = at_d[r0:r0 + 28:9, m0:m0 + MCH]
                        bap = bap.unsqueeze(1).broadcast_to([4, 32, MCH])
                        qeng = [nc.sync, nc.gpsimd, nc.scalar][nd % 3]
                        nd += 1
                        qeng.dma_start(out=bc, in_=bap)
                        bc_t[j][kk] = (bc, None)
                o_sb = osb.tile([128, SUBS, 256], F32, name="o_sb")
                for half in range(2):
                    h0 = half * 512
                    o_gs = [ops.tile([128, 512], F32, name=f"o_g{sub4}")
                            for sub4 in range(4)]
                    for j in range(2):
                        for kk in range(K2):
                            dl = DELTAS[kk]
                            bch = bc_t[j][kk][0][:, h0:h0 + 512]
                            t2 = t2b.tile([128, 512], BF16, name=f"t2_{(j * K2 + kk) % 3}")
                            vsl = vT[j][:, PAD + m0 + h0 + dl:PAD + m0 + h0 + 512 + dl]
                            if _prod_engine('av', ch, j, kk) == 'pool':
                                nc.gpsimd.tensor_mul(t2, bch, vsl)
                            else:
                                nc.vector.tensor_mul(t2, bch, vsl)
                            for sub4 in range(4):
                                nc.tensor.matmul(
                                    o_gs[sub4][:, j * 128:(j + 1) * 128],
                                    t2[:, sub4 * 128:(sub4 + 1) * 128], identb,
                                    start=(kk == 0), stop=(kk == K2 - 1))
                    for sub4 in range(4):
                        sub = half * 4 + sub4
                        for j in range(2):
                            dst = o_sb[:, sub, j * 128:(j + 1) * 128]
                            src = o_gs[sub4][:, j * 128:(j + 1) * 128]
                            nc.scalar.copy(out=dst, in_=src)
                nc.sync.dma_start(
                    out=out_view[:, ch * SUBS:(ch + 1) * SUBS, :, :],
                    in_=o_sb.rearrange("p s (j c) -> p s j c", j=2))

            front(0)
            for ch in range(1, NCH):
                front(ch)
                back(ch - 1)
            back(NCH - 1)
    nc.compile()
    return nc


_NC_CACHE = None


def kernel(x: np.ndarray, W_qkv: np.ndarray) -> np.ndarray:
    global _NC_CACHE
    if _NC_CACHE is None:
        _NC_CACHE = build_nc()
    nc = _NC_CACHE

    x = np.ascontiguousarray(x, dtype=np.float32)
    W_qkv = np.ascontiguousarray(W_qkv, dtype=np.float32)
    ident, identb, ones_s, bkk, maskA, cntT = _host_consts()
    consts = {
        "w": W_qkv, "ident": ident, "identb": identb, "ones_s": ones_s,
        "bkk": bkk, "maskA": maskA, "cntT": cntT,
    }
    in_maps = [
        {"x": x[b].reshape(N, C).copy(), **consts} for b in range(B)
    ]
    res = run_bass_kernel_spmd(nc, in_maps, list(range(B)))
    out = np.stack([res.results[b]["out"].reshape(H, W, C) for b in range(B)])
    return out


if __name__ == "__main__":
    rng = np.random.default_rng(0)
    x = rng.standard_normal((B, H, W, C), dtype=np.float32)
    wq = (rng.standard_normal((3 * C, C), dtype=np.float32) * 0.02).astype(np.float32)
    out = kernel(x, wq)
    print("out", out.shape, out.dtype, float(np.abs(out).mean()))


# revision 5
# speedup vs baseline: 1.6684x; 1.0036x over previous
"""Trainium2 Bass kernel v2 for dilated local attention (nn_DilateAttention).

Problem: x [8, 64, 64, 256] f32, W_qkv [768, 256] f32.
  qkv = x @ W_qkv.T; per pixel, per head (8 heads x 32 dim): attention over
  the 9 dilated (3x3, dilation 3) spatial neighbors with zero padding.

Strategy (data-parallel over batch, 1 image per core), [c, m] on-chip layout:
  - PE: f32r transposes of x/W, f32r qkv projection, per-head score
    reduction with product-as-stationary matmuls, and AV accumulation via
    transpose-accumulate matmuls producing rows-layout output directly.
  - DVE/Pool: the q*k / attn*v elementwise products (bf16, SBUF-only so
    Pool is legal); DVE also runs the softmax chain batched 4 m-subs at a
    time on a single-bank [128, 4, 72] PSUM score tile.
  - Attention normalization (1/den) is applied in [m, 72] layout before
    transposing (free-broadcast scalar_tensor_tensor), then the normalized
    attention is transposed to [72, m] and round-tripped through a DRAM
    scratch so the per-channel broadcast becomes a partition-replicating
    (stride-0) DMA read - no PE/PSUM broadcast or evacuation needed.
  - Emission is software-pipelined: chunk ch+1's products/scores/softmax
    are emitted before chunk ch's AV phase so the in-order engine queues
    interleave the two chunks.
"""

import sys

sys.path.insert(0, "/opt/trn_rl_repo")

import numpy as np
import ml_dtypes
from contextlib import ExitStack

import concourse.bass as bass
import concourse.bacc as bacc
import concourse.tile as tile
from concourse import mybir
from concourse.bass_utils import run_bass_kernel_spmd

B, H, W, C = 8, 64, 64, 256
NH, DPH, K2 = 8, 32, 9
N = H * W          # 4096 pixels
PAD = 256          # zero border on each side of k/v (covers |delta| <= 195)
MCH = 1024         # pixels per m-chunk
NCH = N // MCH     # 4 chunks
SUBS = MCH // 128  # 8 m-subs per chunk
NSUB = N // 128    # 32 m-subs
SCALE = DPH ** -0.5
F32 = mybir.dt.float32
F32R = mybir.dt.float32r
BF16 = mybir.dt.bfloat16
NPBF16 = ml_dtypes.bfloat16

DELTAS = [64 * (3 * i - 3) + (3 * j - 3) for i in range(3) for j in range(3)]


def _prod_engine(phase, ch, j, kk):
    # Pool products are SBUF-only (HW-legal) and cheap in-model.
    if kk in (1, 4, 7):
        return 'pool'
    return 'dve'


def _host_consts():
    ident = np.eye(128, dtype=np.float32)
    identb = np.eye(128, dtype=NPBF16)
    # score reduce (moving operand): ones_s[p, hh] = 1 iff p//32 == hh
    ones_s = np.zeros((128, 4), NPBF16)
    for p in range(128):
        ones_s[p, p // 32] = 1.0
    # kept for interface compat (unused when broadcast is DMA-only)
    bkk = np.zeros((72, 2, 9, 128), NPBF16)
    for jj in range(2):
        for kk in range(9):
            for q in range(128):
                bkk[(4 * jj + q // 32) * 9 + kk, jj, kk, q] = 1.0
    bkk = bkk.reshape(72, 2 * 9 * 128)
    # column-validity 0/1 mask in [m-sub, f=h*9+kk] layout, plus invalid
    # counts for the denominator (reference zero-pads keys: invalid slots
    # contribute exp(0)=1 to the denominator and 0 to the numerator).
    maskA = np.zeros((128, NSUB, 72), NPBF16)
    cntT = np.zeros((128, NSUB), np.float32)
    for ms in range(NSUB):
        m = ms * 128 + np.arange(128)
        jm = m % 64
        for kk in range(9):
            dc = 3 * (kk % 3) - 3
            valid = (((jm + dc) >= 0) & ((jm + dc) < 64)).astype(np.float32)
            for h in range(8):
                maskA[:, ms, h * 9 + kk] = valid
            cntT[:, ms] += 1.0 - valid
    maskA = maskA.reshape(128, NSUB * 72)
    return ident, identb, ones_s, bkk, maskA, cntT


def build_nc() -> bass.Bass:
    nc = bacc.Bacc()
    x_d = nc.declare_dram_parameter("x", [N, C], F32, isOutput=False)
    w_d = nc.declare_dram_parameter("w", [3 * C, C], F32, isOutput=False)
    ident_d = nc.declare_dram_parameter("ident", [128, 128], F32, isOutput=False)
    identb_d = nc.declare_dram_parameter("identb", [128, 128], BF16, isOutput=False)
    ones_s_d = nc.declare_dram_parameter("ones_s", [128, 4], BF16, isOutput=False)
    bkk_d = nc.declare_dram_parameter("bkk", [72, 2 * 9 * 128], BF16, isOutput=False)
    maskA_d = nc.declare_dram_parameter("maskA", [128, NSUB * 72], BF16, isOutput=False)
    cntT_d = nc.declare_dram_parameter("cntT", [128, NSUB], F32, isOutput=False)
    out_d = nc.declare_dram_parameter("out", [N, C], F32, isOutput=True)
    at_d = nc.dram_tensor("at_scratch", [80, N], BF16, kind="Internal")[:, :]

    with tile.TileContext(nc) as tc, ExitStack() as ctx:
        singles = ctx.enter_context(tc.tile_pool(name="singles", bufs=1))
        qkv_pool = ctx.enter_context(tc.tile_pool(name="qkv", bufs=1))

        identr = singles.tile([128, 128], F32R)
        nc.sync.dma_start(out=identr, in_=ident_d[:, :].bitcast(F32R))
        identb = singles.tile([128, 128], BF16)
        nc.sync.dma_start(out=identb, in_=identb_d[:, :])
        ones_s = singles.tile([128, 4], BF16)
        nc.scalar.dma_start(out=ones_s, in_=ones_s_d[:, :])
        maskA = singles.tile([128, NSUB, 72], BF16)
        nc.scalar.dma_start(
            out=maskA, in_=maskA_d[:, :].rearrange("p (s f) -> p s f", f=72))
        cntT = singles.tile([128, NSUB], F32)
        nc.scalar.dma_start(out=cntT, in_=cntT_d[:, :])

        # q/k/v in transposed [c, m] bf16 layout; k/v have zero borders of PAD
        qT = [qkv_pool.tile([128, N], BF16, name=f"qT{j}") for j in range(2)]
        kT = [qkv_pool.tile([128, N + 2 * PAD], BF16, name=f"kT{j}") for j in range(2)]
        vT = [qkv_pool.tile([128, N + 2 * PAD], BF16, name=f"vT{j}") for j in range(2)]
        for j in range(2):
            nc.gpsimd.memset(kT[j][:, 0:PAD], 0.0)
            nc.gpsimd.memset(kT[j][:, PAD + N:], 0.0)
            nc.gpsimd.memset(vT[j][:, 0:PAD], 0.0)
            nc.gpsimd.memset(vT[j][:, PAD + N:], 0.0)

        # normalized attention, [72 rows = h*9+kk, m]
        attn_pool = ctx.enter_context(tc.tile_pool(name="attn_sb", bufs=1))
        at72 = attn_pool.tile([72, N], BF16)

        # ---- P1: W^T tiles and x^T via PE transpose (f32r) ----
        xt_pool = tc.alloc_tile_pool(name="xt_pool", bufs=1)
        with tc.tile_pool(name="trans_sb", bufs=4) as tsb, \
             tc.tile_pool(name="trans_ps", bufs=2, space="PSUM") as tps:
            wlhsT = [singles.tile([128, 6, 128], F32R, name=f"wlhsT{j}") for j in range(2)]
            for ot in range(6):
                w_rows = tsb.tile([128, 256], F32R, name="w_rows")
                nc.scalar.dma_start(out=w_rows, in_=w_d[ot * 128:(ot + 1) * 128, :].bitcast(F32R))
                for j in range(2):
                    wt_ps = tps.tile([128, 128], F32R, name="wt_ps")
                    nc.tensor.transpose(wt_ps, w_rows[:, j * 128:(j + 1) * 128], identr)
                    nc.scalar.copy(out=wlhsT[j][:, ot, :], in_=wt_ps)

            xT = [xt_pool.tile([128, N], F32R, name=f"xT{j}") for j in range(2)]
            xin = x_d[:, :].rearrange("(t p) c -> p t c", p=128).bitcast(F32R)
            for mb in range(8):
                x_rows = tsb.tile([128, 4, 256], F32R, name="x_rows")
                qeng = [nc.sync, nc.gpsimd, nc.scalar][mb % 3]
                qeng.dma_start(out=x_rows, in_=xin[:, mb * 4:(mb + 1) * 4, :])
                for t in range(4):
                    mt = mb * 4 + t
                    xt_ps = tps.tile([128, 256], F32R, name="xt_ps")
                    for j in range(2):
                        nc.tensor.transpose(
                            xt_ps[:, j * 128:(j + 1) * 128],
                            x_rows[:, t, j * 128:(j + 1) * 128], identr)
                    for j in range(2):
                        dst = xT[j][:, mt * 128:(mt + 1) * 128]
                        src = xt_ps[:, j * 128:(j + 1) * 128]
                        if (mt * 2 + j) % 4 != 3:
                            nc.vector.tensor_copy(out=dst, in_=src)
                        else:
                            nc.scalar.copy(out=dst, in_=src)

        # ---- P2: qkv projection (f32r) -> bf16 qT/kT/vT ----
        with tc.tile_pool(name="qkv_ps", bufs=4, space="PSUM") as qps:
            for ot in range(6):
                for ch in range(8):
                    acc = qps.tile([128, 512], F32, name="acc")
                    for j in range(2):
                        nc.tensor.matmul(
                            acc, wlhsT[j][:, ot, :],
                            xT[j][:, ch * 512:(ch + 1) * 512],
                            start=(j == 0), stop=(j == 1))
                    dst_j = ot % 2
                    if ot < 2:
                        dst = qT[dst_j][:, ch * 512:(ch + 1) * 512]
                    elif ot < 4:
                        dst = kT[dst_j][:, PAD + ch * 512:PAD + (ch + 1) * 512]
                    else:
                        dst = vT[dst_j][:, PAD + ch * 512:PAD + (ch + 1) * 512]
                    if (ot * 8 + ch) % 3 == 0:
                        nc.scalar.copy(out=dst, in_=acc)
                    else:
                        nc.vector.tensor_copy(out=dst, in_=acc)
        xt_pool.release()

        # ---- P3/P4, software-pipelined over m-chunks ----
        out_view = out_d[:, :].rearrange(
            "(s p) (j c) -> p s j c", p=128, j=2)

        with tc.tile_pool(name="prod_sb", bufs=1) as ttb, \
             tc.tile_pool(name="t2_sb", bufs=2) as t2b, \
             tc.tile_pool(name="bcd_sb", bufs=1) as bsb, \
             tc.tile_pool(name="sm_sb", bufs=4) as smb, \
             tc.tile_pool(name="sc_ps", bufs=2, space="PSUM") as sps, \
             tc.tile_pool(name="at_ps", bufs=2, space="PSUM") as aps, \
             tc.tile_pool(name="o_ps", bufs=1, space="PSUM") as ops, \
             tc.tile_pool(name="o_sb", bufs=2) as osb:

            def front(ch):
                """products -> scores -> softmax -> normalized at72 -> DRAM."""
                m0 = ch * MCH
                t_t = [[None] * K2 for _ in range(2)]
                for kk in range(K2):
                    dl = DELTAS[kk]
                    for j in range(2):
                        t = ttb.tile([128, MCH], BF16, name=f"pr{j}_{kk}")
                        if _prod_engine('qk', ch, j, kk) == 'pool':
                            nc.gpsimd.tensor_mul(
                                t, qT[j][:, m0:m0 + MCH],
                                kT[j][:, PAD + m0 + dl:PAD + m0 + MCH + dl])
                        else:
                            nc.vector.tensor_mul(
                                t, qT[j][:, m0:m0 + MCH],
                                kT[j][:, PAD + m0 + dl:PAD + m0 + MCH + dl])
                        t_t[j][kk] = t
                for g in range(SUBS // 4):
                    s_ps = sps.tile([128, 4, 72], F32, name="s_ps")
                    for sub4 in range(4):
                        sub = g * 4 + sub4
                        for kk in range(K2):
                            for j in range(2):
                                out_ap = s_ps.rearrange(
                                    "p s (h k) -> p s h k", k=9)[:, sub4, 4 * j:4 * j + 4, kk]
                                nc.tensor.matmul(
                                    out_ap,
                                    t_t[j][kk][:, sub * 128:sub * 128 + 128],
                                    ones_s, start=True, stop=True)
                    ms0 = ch * SUBS + g * 4
                    em0 = smb.tile([128, 4, 72], BF16, name="em0")
                    nc.scalar.activation(
                        em0, s_ps, mybir.ActivationFunctionType.Exp,
                        scale=float(SCALE))
                    em = smb.tile([128, 4, 72], BF16, name="em")
                    nc.vector.tensor_mul(em, em0, maskA[:, ms0:ms0 + 4, :])
                    den = smb.tile([128, 4, 8], F32, name="den")
                    nc.vector.reduce_sum(
                        den, em.rearrange("p s (h k) -> p s h k", k=9),
                        axis=mybir.AxisListType.X)
                    cb = cntT[:, ms0:ms0 + 4].unsqueeze(2).broadcast_to([128, 4, 8])
                    nc.vector.scalar_tensor_tensor(
                        out=den, in0=den, scalar=1.0, in1=cb,
                        op0=mybir.AluOpType.mult, op1=mybir.AluOpType.add)
                    rr = smb.tile([128, 4, 8], F32, name="rr")
                    nc.vector.reciprocal(rr, den)
                    rrb = smb.tile([128, 4, 8], BF16, name="rrb")
                    nc.vector.tensor_copy(out=rrb, in_=rr)
                    emn = smb.tile([128, 4, 72], BF16, name="emn")
                    rbc = rrb[:, :, :].unsqueeze(3).broadcast_to([128, 4, 8, 9])
                    nc.vector.scalar_tensor_tensor(
                        out=emn.rearrange("p s (h k) -> p s h k", k=9),
                        in0=em.rearrange("p s (h k) -> p s h k", k=9),
                        scalar=1.0, in1=rbc,
                        op0=mybir.AluOpType.mult, op1=mybir.AluOpType.mult)
                    at_ps = aps.tile([72, 4, 128], F32, name="at_ps")
                    for sub4 in range(4):
                        nc.tensor.matmul(
                            at_ps[:, sub4, :], emn[:, sub4, :], identb,
                            start=True, stop=True)
                    nc.scalar.copy(
                        out=at72[:, m0 + g * 512:m0 + (g + 1) * 512],
                        in_=at_ps.rearrange("p s q -> p (s q)"))
                nc.sync.dma_start(out=at_d[0:72, m0:m0 + MCH],
                                  in_=at72[:, m0:m0 + MCH])

            def back(ch):
                """DMA-broadcast attn, attn*v products, transpose-accumulate."""
                m0 = ch * MCH
                bc_t = [[None] * K2 for _ in range(2)]
                nd = 0
                for kk in range(K2):
                    for j in range(2):
                        bc = bsb.tile([128, MCH], BF16, name=f"bcd{j}_{kk}")
                        r0 = (4 * j) * 9 + kk
                        bap = at_d[r0:r0 + 28:9, m0:m0 + MCH]
                        bap = bap.unsqueeze(1).broadcast_to([4, 32, MCH])
                        qeng = [nc.sync, nc.gpsimd, nc.scalar][nd % 3]
                        nd += 1
                        qeng.dma_start(out=bc, in_=bap)
                        bc_t[j][kk] = (bc, None)
                o_sb = osb.tile([128, SUBS, 256], F32, name="o_sb")
                for half in range(2):
                    h0 = half * 512
                    o_gs = [ops.tile([128, 512], F32, name=f"o_g{sub4}")
                            for sub4 in range(4)]
                    for j in range(2):
                        for kk in range(K2):
                            dl = DELTAS[kk]
                            bch = bc_t[j][kk][0][:, h0:h0 + 512]
                            t2 = t2b.tile([128, 512], BF16, name=f"t2_{(j * K2 + kk) % 3}")
                            vsl = vT[j][:, PAD + m0 + h0 + dl:PAD + m0 + h0 + 512 + dl]
                            if _prod_engine('av', ch, j, kk) == 'pool':
                                nc.gpsimd.tensor_mul(t2, bch, vsl)
                            else:
                                nc.vector.tensor_mul(t2, bch, vsl)
                            for sub4 in range(4):
                                nc.tensor.matmul(
                                    o_gs[sub4][:, j * 128:(j + 1) * 128],
                                    t2[:, sub4 * 128:(sub4 + 1) * 128], identb,
                                    start=(kk == 0), stop=(kk == K2 - 1))
                    for sub4 in range(4):
                        sub = half * 4 + sub4
                        for j in range(2):
                            dst = o_sb[:, sub, j * 128:(j + 1) * 128]
                            src = o_gs[sub4][:, j * 128:(j + 1) * 128]
                            nc.scalar.copy(out=dst, in_=src)
                nc.sync.dma_start(
                    out=out_view[:, ch * SUBS:(ch + 1) * SUBS, :, :],
                    in_=o_sb.rearrange("p s (j c) -> p s j c", j=2))

            front(0)
            for ch in range(1, NCH):
                front(ch)
                back(ch - 1)
            back(NCH - 1)
    nc.compile()
    return nc


_NC_CACHE = None


def kernel(x: np.ndarray, W_qkv: np.ndarray) -> np.ndarray:
    global _NC_CACHE
    if _NC_CACHE is None:
        _NC_CACHE = build_nc()
    nc = _NC_CACHE

    x = np.ascontiguousarray(x, dtype=np.float32)
    W_qkv = np.ascontiguousarray(W_qkv, dtype=np.float32)
    ident, identb, ones_s, bkk, maskA, cntT = _host_consts()
    consts = {
        "w": W_qkv, "ident": ident, "identb": identb, "ones_s": ones_s,
        "bkk": bkk, "maskA": maskA, "cntT": cntT,
    }
    in_maps = [
        {"x": x[b].reshape(N, C).copy(), **consts} for b in range(B)
    ]
    res = run_bass_kernel_spmd(nc, in_maps, list(range(B)))
    out = np.stack([res.results[b]["out"].reshape(H, W, C) for b in range(B)])
    return out


if __name__ == "__main__":
    rng = np.random.default_rng(0)
    x = rng.standard_normal((B, H, W, C), dtype=np.float32)
    wq = (rng.standard_normal((3 * C, C), dtype=np.float32) * 0.02).astype(np.float32)
    out = kernel(x, wq)
    print("out", out.shape, out.dtype, float(np.abs(out).mean()))


# revision 6
# speedup vs baseline: 1.6970x; 1.0172x over previous
"""Trainium2 Bass kernel v2 for dilated local attention (nn_DilateAttention).

Problem: x [8, 64, 64, 256] f32, W_qkv [768, 256] f32.
  qkv = x @ W_qkv.T; per pixel, per head (8 heads x 32 dim): attention over
  the 9 dilated (3x3, dilation 3) spatial neighbors with zero padding.

Strategy (data-parallel over batch, 1 image per core), [c, m] on-chip layout:
  - PE: f32r transposes of x/W, f32r qkv projection, per-head score
    reduction with product-as-stationary matmuls, and AV accumulation via
    transpose-accumulate matmuls producing rows-layout output directly.
  - DVE/Pool: the q*k / attn*v elementwise products (bf16, SBUF-only so
    Pool is legal); DVE also runs the softmax chain batched 4 m-subs at a
    time on a single-bank [128, 4, 72] PSUM score tile.
  - Attention normalization (1/den) is applied in [m, 72] layout before
    transposing (free-broadcast scalar_tensor_tensor), then the normalized
    attention is transposed to [72, m] and round-tripped through a DRAM
    scratch so the per-channel broadcast becomes a partition-replicating
    (stride-0) DMA read - no PE/PSUM broadcast or evacuation needed.
  - Emission is software-pipelined: chunk ch+1's products/scores/softmax
    are emitted before chunk ch's AV phase so the in-order engine queues
    interleave the two chunks.
"""

import sys

sys.path.insert(0, "/opt/trn_rl_repo")

import numpy as np
import ml_dtypes
from contextlib import ExitStack

import concourse.bass as bass
import concourse.bacc as bacc
import concourse.tile as tile
from concourse import mybir
from concourse.bass_utils import run_bass_kernel_spmd

B, H, W, C = 8, 64, 64, 256
NH, DPH, K2 = 8, 32, 9
N = H * W          # 4096 pixels
PAD = 256          # zero border on each side of k/v (covers |delta| <= 195)
MCH = 1024         # pixels per m-chunk
NCH = N // MCH     # 4 chunks
SUBS = MCH // 128  # 8 m-subs per chunk
NSUB = N // 128    # 32 m-subs
SCALE = DPH ** -0.5
F32 = mybir.dt.float32
F32R = mybir.dt.float32r
BF16 = mybir.dt.bfloat16
NPBF16 = ml_dtypes.bfloat16

DELTAS = [64 * (3 * i - 3) + (3 * j - 3) for i in range(3) for j in range(3)]


def _prod_engine(phase, ch, j, kk):
    # Pool products are SBUF-only (HW-legal) and cheap in-model.
    if kk in (1, 4, 7):
        return 'pool'
    return 'dve'


def _host_consts():
    ident = np.eye(128, dtype=np.float32)
    identb = np.eye(128, dtype=NPBF16)
    # score reduce (moving operand): ones_s[p, hh] = 1 iff p//32 == hh
    ones_s = np.zeros((128, 4), NPBF16)
    for p in range(128):
        ones_s[p, p // 32] = 1.0
    # kept for interface compat (unused when broadcast is DMA-only)
    bkk = np.zeros((72, 2, 9, 128), NPBF16)
    for jj in range(2):
        for kk in range(9):
            for q in range(128):
                bkk[(4 * jj + q // 32) * 9 + kk, jj, kk, q] = 1.0
    bkk = bkk.reshape(72, 2 * 9 * 128)
    # column-validity 0/1 mask in [m-sub, f=h*9+kk] layout, plus invalid
    # counts for the denominator (reference zero-pads keys: invalid slots
    # contribute exp(0)=1 to the denominator and 0 to the numerator).
    maskA = np.zeros((128, NSUB, 72), NPBF16)
    cntT = np.zeros((128, NSUB), np.float32)
    for ms in range(NSUB):
        m = ms * 128 + np.arange(128)
        jm = m % 64
        for kk in range(9):
            dc = 3 * (kk % 3) - 3
            valid = (((jm + dc) >= 0) & ((jm + dc) < 64)).astype(np.float32)
            for h in range(8):
                maskA[:, ms, h * 9 + kk] = valid
            cntT[:, ms] += 1.0 - valid
    maskA = maskA.reshape(128, NSUB * 72)
    return ident, identb, ones_s, bkk, maskA, cntT


def build_nc() -> bass.Bass:
    nc = bacc.Bacc()
    x_d = nc.declare_dram_parameter("x", [N, C], F32, isOutput=False)
    w_d = nc.declare_dram_parameter("w", [3 * C, C], F32, isOutput=False)
    ident_d = nc.declare_dram_parameter("ident", [128, 128], F32, isOutput=False)
    identb_d = nc.declare_dram_parameter("identb", [128, 128], BF16, isOutput=False)
    ones_s_d = nc.declare_dram_parameter("ones_s", [128, 4], BF16, isOutput=False)
    bkk_d = nc.declare_dram_parameter("bkk", [72, 2 * 9 * 128], BF16, isOutput=False)
    maskA_d = nc.declare_dram_parameter("maskA", [128, NSUB * 72], BF16, isOutput=False)
    cntT_d = nc.declare_dram_parameter("cntT", [128, NSUB], F32, isOutput=False)
    out_d = nc.declare_dram_parameter("out", [N, C], F32, isOutput=True)
    at_d = nc.dram_tensor("at_scratch", [80, N], BF16, kind="Internal")[:, :]

    with tile.TileContext(nc) as tc, ExitStack() as ctx:
        singles = ctx.enter_context(tc.tile_pool(name="singles", bufs=1))
        qkv_pool = ctx.enter_context(tc.tile_pool(name="qkv", bufs=1))

        identr = singles.tile([128, 128], F32R)
        nc.sync.dma_start(out=identr, in_=ident_d[:, :].bitcast(F32R))
        identb = singles.tile([128, 128], BF16)
        nc.sync.dma_start(out=identb, in_=identb_d[:, :])
        ones_s = singles.tile([128, 4], BF16)
        nc.scalar.dma_start(out=ones_s, in_=ones_s_d[:, :])
        maskA = singles.tile([128, NSUB, 72], BF16)
        nc.scalar.dma_start(
            out=maskA, in_=maskA_d[:, :].rearrange("p (s f) -> p s f", f=72))
        cntT = singles.tile([128, NSUB], F32)
        nc.scalar.dma_start(out=cntT, in_=cntT_d[:, :])

        # q/k/v in transposed [c, m] bf16 layout; k/v have zero borders of PAD
        qT = [qkv_pool.tile([128, N], BF16, name=f"qT{j}") for j in range(2)]
        kT = [qkv_pool.tile([128, N + 2 * PAD], BF16, name=f"kT{j}") for j in range(2)]
        vT = [qkv_pool.tile([128, N + 2 * PAD], BF16, name=f"vT{j}") for j in range(2)]
        for j in range(2):
            nc.gpsimd.memset(kT[j][:, 0:PAD], 0.0)
            nc.gpsimd.memset(kT[j][:, PAD + N:], 0.0)
            nc.gpsimd.memset(vT[j][:, 0:PAD], 0.0)
            nc.gpsimd.memset(vT[j][:, PAD + N:], 0.0)

        # normalized attention, [72 rows = h*9+kk, m]
        attn_pool = ctx.enter_context(tc.tile_pool(name="attn_sb", bufs=1))
        at72 = attn_pool.tile([72, N], BF16)

        # ---- P1: W^T tiles and x^T via PE transpose (f32r) ----
        xt_pool = tc.alloc_tile_pool(name="xt_pool", bufs=1)
        with tc.tile_pool(name="trans_sb", bufs=4) as tsb, \
             tc.tile_pool(name="trans_ps", bufs=2, space="PSUM") as tps:
            wlhsT = [singles.tile([128, 6, 128], F32R, name=f"wlhsT{j}") for j in range(2)]
            for ot in range(6):
                w_rows = tsb.tile([128, 256], F32R, name="w_rows")
                nc.scalar.dma_start(out=w_rows, in_=w_d[ot * 128:(ot + 1) * 128, :].bitcast(F32R))
                for j in range(2):
                    wt_ps = tps.tile([128, 128], F32R, name="wt_ps")
                    nc.tensor.transpose(wt_ps, w_rows[:, j * 128:(j + 1) * 128], identr)
                    nc.scalar.copy(out=wlhsT[j][:, ot, :], in_=wt_ps)

            xT = [xt_pool.tile([128, N], F32R, name=f"xT{j}") for j in range(2)]
            xin = x_d[:, :].rearrange("(t p) c -> p t c", p=128).bitcast(F32R)
            for mb in range(8):
                x_rows = tsb.tile([128, 4, 256], F32R, name="x_rows")
                qeng = [nc.sync, nc.gpsimd, nc.scalar][mb % 3]
                qeng.dma_start(out=x_rows, in_=xin[:, mb * 4:(mb + 1) * 4, :])
                for t in range(4):
                    mt = mb * 4 + t
                    xt_ps = tps.tile([128, 256], F32R, name="xt_ps")
                    for j in range(2):
                        nc.tensor.transpose(
                            xt_ps[:, j * 128:(j + 1) * 128],
                            x_rows[:, t, j * 128:(j + 1) * 128], identr)
                    for j in range(2):
                        dst = xT[j][:, mt * 128:(mt + 1) * 128]
                        src = xt_ps[:, j * 128:(j + 1) * 128]
                        if (mt * 2 + j) % 4 != 3:
                            nc.vector.tensor_copy(out=dst, in_=src)
                        else:
                            nc.scalar.copy(out=dst, in_=src)

        # ---- P2: qkv projection (f32r) -> bf16 qT/kT/vT ----
        with tc.tile_pool(name="qkv_ps", bufs=4, space="PSUM") as qps:
            for ot in range(6):
                for ch in range(8):
                    acc = qps.tile([128, 512], F32, name="acc")
                    for j in range(2):
                        nc.tensor.matmul(
                            acc, wlhsT[j][:, ot, :],
                            xT[j][:, ch * 512:(ch + 1) * 512],
                            start=(j == 0), stop=(j == 1))
                    dst_j = ot % 2
                    if ot < 2:
                        dst = qT[dst_j][:, ch * 512:(ch + 1) * 512]
                    elif ot < 4:
                        dst = kT[dst_j][:, PAD + ch * 512:PAD + (ch + 1) * 512]
                    else:
                        dst = vT[dst_j][:, PAD + ch * 512:PAD + (ch + 1) * 512]
                    if (ot * 8 + ch) % 3 == 0:
                        nc.scalar.copy(out=dst, in_=acc)
                    else:
                        nc.vector.tensor_copy(out=dst, in_=acc)
        xt_pool.release()

        # ---- P3/P4, software-pipelined over m-chunks ----
        out_view = out_d[:, :].rearrange(
            "(s p) (j c) -> p s j c", p=128, j=2)

        with tc.tile_pool(name="prod_sb", bufs=1) as ttb, \
             tc.tile_pool(name="t2_sb", bufs=4) as t2b, \
             tc.tile_pool(name="bcd_sb", bufs=1) as bsb, \
             tc.tile_pool(name="sm_sb", bufs=4) as smb, \
             tc.tile_pool(name="sc_ps", bufs=2, space="PSUM") as sps, \
             tc.tile_pool(name="at_ps", bufs=2, space="PSUM") as aps, \
             tc.tile_pool(name="o_ps", bufs=1, space="PSUM") as ops, \
             tc.tile_pool(name="o_sb", bufs=2) as osb:

            def front(ch):
                """products -> scores -> softmax -> normalized at72 -> DRAM."""
                m0 = ch * MCH
                t_t = [[None] * K2 for _ in range(2)]
                for kk in range(K2):
                    dl = DELTAS[kk]
                    for j in range(2):
                        t = ttb.tile([128, MCH], BF16, name=f"pr{j}_{kk}")
                        if _prod_engine('qk', ch, j, kk) == 'pool':
                            nc.gpsimd.tensor_mul(
                                t, qT[j][:, m0:m0 + MCH],
                                kT[j][:, PAD + m0 + dl:PAD + m0 + MCH + dl])
                        else:
                            nc.vector.tensor_mul(
                                t, qT[j][:, m0:m0 + MCH],
                                kT[j][:, PAD + m0 + dl:PAD + m0 + MCH + dl])
                        t_t[j][kk] = t
                for g in range(SUBS // 4):
                    s_ps = sps.tile([128, 4, 72], F32, name="s_ps")
                    for sub4 in range(4):
                        sub = g * 4 + sub4
                        for kk in range(K2):
                            for j in range(2):
                                out_ap = s_ps.rearrange(
                                    "p s (h k) -> p s h k", k=9)[:, sub4, 4 * j:4 * j + 4, kk]
                                nc.tensor.matmul(
                                    out_ap,
                                    t_t[j][kk][:, sub * 128:sub * 128 + 128],
                                    ones_s, start=True, stop=True)
                    ms0 = ch * SUBS + g * 4
                    em0 = smb.tile([128, 4, 72], BF16, name="em0")
                    nc.scalar.activation(
                        em0, s_ps, mybir.ActivationFunctionType.Exp,
                        scale=float(SCALE))
                    em = smb.tile([128, 4, 72], BF16, name="em")
                    nc.vector.tensor_mul(em, em0, maskA[:, ms0:ms0 + 4, :])
                    den = smb.tile([128, 4, 8], F32, name="den")
                    nc.vector.reduce_sum(
                        den, em.rearrange("p s (h k) -> p s h k", k=9),
                        axis=mybir.AxisListType.X)
                    cb = cntT[:, ms0:ms0 + 4].unsqueeze(2).broadcast_to([128, 4, 8])
                    nc.vector.scalar_tensor_tensor(
                        out=den, in0=den, scalar=1.0, in1=cb,
                        op0=mybir.AluOpType.mult, op1=mybir.AluOpType.add)
                    rr = smb.tile([128, 4, 8], F32, name="rr")
                    nc.vector.reciprocal(rr, den)
                    rrb = smb.tile([128, 4, 8], BF16, name="rrb")
                    nc.vector.tensor_copy(out=rrb, in_=rr)
                    emn = smb.tile([128, 4, 72], BF16, name="emn")
                    rbc = rrb[:, :, :].unsqueeze(3).broadcast_to([128, 4, 8, 9])
                    nc.vector.scalar_tensor_tensor(
                        out=emn.rearrange("p s (h k) -> p s h k", k=9),
                        in0=em.rearrange("p s (h k) -> p s h k", k=9),
                        scalar=1.0, in1=rbc,
                        op0=mybir.AluOpType.mult, op1=mybir.AluOpType.mult)
                    at_ps = aps.tile([72, 4, 128], F32, name="at_ps")
                    for sub4 in range(4):
                        nc.tensor.matmul(
                            at_ps[:, sub4, :], emn[:, sub4, :], identb,
                            start=True, stop=True)
                    nc.scalar.copy(
                        out=at72[:, m0 + g * 512:m0 + (g + 1) * 512],
                        in_=at_ps.rearrange("p s q -> p (s q)"))
                nc.sync.dma_start(out=at_d[0:72, m0:m0 + MCH],
                                  in_=at72[:, m0:m0 + MCH])

            def back(ch):
                """DMA-broadcast attn, attn*v products, transpose-accumulate."""
                m0 = ch * MCH
                bc_t = [[None] * K2 for _ in range(2)]
                nd = 0
                for kk in range(K2):
                    for j in range(2):
                        bc = bsb.tile([128, MCH], BF16, name=f"bcd{j}_{kk}")
                        r0 = (4 * j) * 9 + kk
                        bap = at_d[r0:r0 + 28:9, m0:m0 + MCH]
                        bap = bap.unsqueeze(1).broadcast_to([4, 32, MCH])
                        qeng = [nc.sync, nc.gpsimd, nc.scalar][nd % 3]
                        nd += 1
                        qeng.dma_start(out=bc, in_=bap)
                        bc_t[j][kk] = (bc, None)
                o_sb = osb.tile([128, SUBS, 256], F32, name="o_sb")
                for half in range(2):
                    h0 = half * 512
                    o_gs = [ops.tile([128, 512], F32, name=f"o_g{sub4}")
                            for sub4 in range(4)]
                    for j in range(2):
                        for kk in range(K2):
                            dl = DELTAS[kk]
                            bch = bc_t[j][kk][0][:, h0:h0 + 512]
                            t2 = t2b.tile([128, 512], BF16, name=f"t2_{(j * K2 + kk) % 3}")
                            vsl = vT[j][:, PAD + m0 + h0 + dl:PAD + m0 + h0 + 512 + dl]
                            if _prod_engine('av', ch, j, kk) == 'pool':
                                nc.gpsimd.tensor_mul(t2, bch, vsl)
                            else:
                                nc.vector.tensor_mul(t2, bch, vsl)
                            for sub4 in range(4):
                                nc.tensor.matmul(
                                    o_gs[sub4][:, j * 128:(j + 1) * 128],
                                    t2[:, sub4 * 128:(sub4 + 1) * 128], identb,
                                    start=(kk == 0), stop=(kk == K2 - 1))
                    for sub4 in range(4):
                        sub = half * 4 + sub4
                        for j in range(2):
                            dst = o_sb[:, sub, j * 128:(j + 1) * 128]
                            src = o_gs[sub4][:, j * 128:(j + 1) * 128]
                            nc.scalar.copy(out=dst, in_=src)
                nc.sync.dma_start(
                    out=out_view[:, ch * SUBS:(ch + 1) * SUBS, :, :],
                    in_=o_sb.rearrange("p s (j c) -> p s j c", j=2))

            front(0)
            for ch in range(1, NCH):
                front(ch)
                back(ch - 1)
            back(NCH - 1)
    nc.compile()
    return nc


_NC_CACHE = None


def kernel(x: np.ndarray, W_qkv: np.ndarray) -> np.ndarray:
    global _NC_CACHE
    if _NC_CACHE is None:
        _NC_CACHE = build_nc()
    nc = _NC_CACHE

    x = np.ascontiguousarray(x, dtype=np.float32)
    W_qkv = np.ascontiguousarray(W_qkv, dtype=np.float32)
    ident, identb, ones_s, bkk, maskA, cntT = _host_consts()
    consts = {
        "w": W_qkv, "ident": ident, "identb": identb, "ones_s": ones_s,
        "bkk": bkk, "maskA": maskA, "cntT": cntT,
    }
    in_maps = [
        {"x": x[b].reshape(N, C).copy(), **consts} for b in range(B)
    ]
    res = run_bass_kernel_spmd(nc, in_maps, list(range(B)))
    out = np.stack([res.results[b]["out"].reshape(H, W, C) for b in range(B)])
    return out


if __name__ == "__main__":
    rng = np.random.default_rng(0)
    x = rng.standard_normal((B, H, W, C), dtype=np.float32)
    wq = (rng.standard_normal((3 * C, C), dtype=np.float32) * 0.02).astype(np.float32)
    out = kernel(x, wq)
    print("out", out.shape, out.dtype, float(np.abs(out).mean()))


# revision 7
# speedup vs baseline: 1.7095x; 1.0073x over previous
"""Trainium2 Bass kernel v2 for dilated local attention (nn_DilateAttention).

Problem: x [8, 64, 64, 256] f32, W_qkv [768, 256] f32.
  qkv = x @ W_qkv.T; per pixel, per head (8 heads x 32 dim): attention over
  the 9 dilated (3x3, dilation 3) spatial neighbors with zero padding.

Strategy (data-parallel over batch, 1 image per core), [c, m] on-chip layout:
  - PE: f32r transposes of x/W, f32r qkv projection, per-head score
    reduction with product-as-stationary matmuls, and AV accumulation via
    transpose-accumulate matmuls producing rows-layout output directly.
  - DVE/Pool: the q*k / attn*v elementwise products (bf16, SBUF-only so
    Pool is legal); DVE also runs the softmax chain batched 4 m-subs at a
    time on a single-bank [128, 4, 72] PSUM score tile.
  - Attention normalization (1/den) is applied in [m, 72] layout before
    transposing (free-broadcast scalar_tensor_tensor), then the normalized
    attention is transposed to [72, m] and round-tripped through a DRAM
    scratch so the per-channel broadcast becomes a partition-replicating
    (stride-0) DMA read - no PE/PSUM broadcast or evacuation needed.
  - Emission is software-pipelined: chunk ch+1's products/scores/softmax
    are emitted before chunk ch's AV phase so the in-order engine queues
    interleave the two chunks.
"""

import sys

sys.path.insert(0, "/opt/trn_rl_repo")

import numpy as np
import ml_dtypes
from contextlib import ExitStack

import concourse.bass as bass
import concourse.bacc as bacc
import concourse.tile as tile
from concourse import mybir
from concourse.bass_utils import run_bass_kernel_spmd

B, H, W, C = 8, 64, 64, 256
NH, DPH, K2 = 8, 32, 9
N = H * W          # 4096 pixels
PAD = 256          # zero border on each side of k/v (covers |delta| <= 195)
MCH = 1024         # pixels per m-chunk
NCH = N // MCH     # 4 chunks
SUBS = MCH // 128  # 8 m-subs per chunk
NSUB = N // 128    # 32 m-subs
SCALE = DPH ** -0.5
F32 = mybir.dt.float32
F32R = mybir.dt.float32r
BF16 = mybir.dt.bfloat16
NPBF16 = ml_dtypes.bfloat16

DELTAS = [64 * (3 * i - 3) + (3 * j - 3) for i in range(3) for j in range(3)]


def _prod_engine(phase, ch, j, kk):
    # Pool products are SBUF-only (HW-legal) and cheap in-model.
    if kk in (1, 4, 7):
        return 'pool'
    if kk == 0 and j == 1:
        return 'pool'
    return 'dve'


def _host_consts():
    ident = np.eye(128, dtype=np.float32)
    identb = np.eye(128, dtype=NPBF16)
    # score reduce (moving operand): ones_s[p, hh] = 1 iff p//32 == hh
    ones_s = np.zeros((128, 4), NPBF16)
    for p in range(128):
        ones_s[p, p // 32] = 1.0
    # kept for interface compat (unused when broadcast is DMA-only)
    bkk = np.zeros((72, 2, 9, 128), NPBF16)
    for jj in range(2):
        for kk in range(9):
            for q in range(128):
                bkk[(4 * jj + q // 32) * 9 + kk, jj, kk, q] = 1.0
    bkk = bkk.reshape(72, 2 * 9 * 128)
    # column-validity 0/1 mask in [m-sub, f=h*9+kk] layout, plus invalid
    # counts for the denominator (reference zero-pads keys: invalid slots
    # contribute exp(0)=1 to the denominator and 0 to the numerator).
    maskA = np.zeros((128, NSUB, 72), NPBF16)
    cntT = np.zeros((128, NSUB), np.float32)
    for ms in range(NSUB):
        m = ms * 128 + np.arange(128)
        jm = m % 64
        for kk in range(9):
            dc = 3 * (kk % 3) - 3
            valid = (((jm + dc) >= 0) & ((jm + dc) < 64)).astype(np.float32)
            for h in range(8):
                maskA[:, ms, h * 9 + kk] = valid
            cntT[:, ms] += 1.0 - valid
    maskA = maskA.reshape(128, NSUB * 72)
    return ident, identb, ones_s, bkk, maskA, cntT


def build_nc() -> bass.Bass:
    nc = bacc.Bacc()
    x_d = nc.declare_dram_parameter("x", [N, C], F32, isOutput=False)
    w_d = nc.declare_dram_parameter("w", [3 * C, C], F32, isOutput=False)
    ident_d = nc.declare_dram_parameter("ident", [128, 128], F32, isOutput=False)
    identb_d = nc.declare_dram_parameter("identb", [128, 128], BF16, isOutput=False)
    ones_s_d = nc.declare_dram_parameter("ones_s", [128, 4], BF16, isOutput=False)
    bkk_d = nc.declare_dram_parameter("bkk", [72, 2 * 9 * 128], BF16, isOutput=False)
    maskA_d = nc.declare_dram_parameter("maskA", [128, NSUB * 72], BF16, isOutput=False)
    cntT_d = nc.declare_dram_parameter("cntT", [128, NSUB], F32, isOutput=False)
    out_d = nc.declare_dram_parameter("out", [N, C], F32, isOutput=True)
    at_d = nc.dram_tensor("at_scratch", [80, N], BF16, kind="Internal")[:, :]

    with tile.TileContext(nc) as tc, ExitStack() as ctx:
        singles = ctx.enter_context(tc.tile_pool(name="singles", bufs=1))
        qkv_pool = ctx.enter_context(tc.tile_pool(name="qkv", bufs=1))

        identr = singles.tile([128, 128], F32R)
        nc.sync.dma_start(out=identr, in_=ident_d[:, :].bitcast(F32R))
        identb = singles.tile([128, 128], BF16)
        nc.sync.dma_start(out=identb, in_=identb_d[:, :])
        ones_s = singles.tile([128, 4], BF16)
        nc.scalar.dma_start(out=ones_s, in_=ones_s_d[:, :])
        maskA = singles.tile([128, NSUB, 72], BF16)
        nc.scalar.dma_start(
            out=maskA, in_=maskA_d[:, :].rearrange("p (s f) -> p s f", f=72))
        cntT = singles.tile([128, NSUB], F32)
        nc.scalar.dma_start(out=cntT, in_=cntT_d[:, :])

        # q/k/v in transposed [c, m] bf16 layout; k/v have zero borders of PAD
        qT = [qkv_pool.tile([128, N], BF16, name=f"qT{j}") for j in range(2)]
        kT = [qkv_pool.tile([128, N + 2 * PAD], BF16, name=f"kT{j}") for j in range(2)]
        vT = [qkv_pool.tile([128, N + 2 * PAD], BF16, name=f"vT{j}") for j in range(2)]
        for j in range(2):
            nc.gpsimd.memset(kT[j][:, 0:PAD], 0.0)
            nc.gpsimd.memset(kT[j][:, PAD + N:], 0.0)
            nc.gpsimd.memset(vT[j][:, 0:PAD], 0.0)
            nc.gpsimd.memset(vT[j][:, PAD + N:], 0.0)

        # normalized attention, [72 rows = h*9+kk, m]
        attn_pool = ctx.enter_context(tc.tile_pool(name="attn_sb", bufs=1))
        at72 = attn_pool.tile([72, N], BF16)

        # ---- P1: W^T tiles and x^T via PE transpose (f32r) ----
        xt_pool = tc.alloc_tile_pool(name="xt_pool", bufs=1)
        with tc.tile_pool(name="trans_sb", bufs=4) as tsb, \
             tc.tile_pool(name="trans_ps", bufs=2, space="PSUM") as tps:
            wlhsT = [singles.tile([128, 6, 128], F32R, name=f"wlhsT{j}") for j in range(2)]
            for ot in range(6):
                w_rows = tsb.tile([128, 256], F32R, name="w_rows")
                nc.scalar.dma_start(out=w_rows, in_=w_d[ot * 128:(ot + 1) * 128, :].bitcast(F32R))
                for j in range(2):
                    wt_ps = tps.tile([128, 128], F32R, name="wt_ps")
                    nc.tensor.transpose(wt_ps, w_rows[:, j * 128:(j + 1) * 128], identr)
                    nc.scalar.copy(out=wlhsT[j][:, ot, :], in_=wt_ps)

            xT = [xt_pool.tile([128, N], F32R, name=f"xT{j}") for j in range(2)]
            xin = x_d[:, :].rearrange("(t p) c -> p t c", p=128).bitcast(F32R)
            for mb in range(8):
                x_rows = tsb.tile([128, 4, 256], F32R, name="x_rows")
                qeng = [nc.sync, nc.gpsimd, nc.scalar][mb % 3]
                qeng.dma_start(out=x_rows, in_=xin[:, mb * 4:(mb + 1) * 4, :])
                for t in range(4):
                    mt = mb * 4 + t
                    xt_ps = tps.tile([128, 256], F32R, name="xt_ps")
                    for j in range(2):
                        nc.tensor.transpose(
                            xt_ps[:, j * 128:(j + 1) * 128],
                            x_rows[:, t, j * 128:(j + 1) * 128], identr)
                    for j in range(2):
                        dst = xT[j][:, mt * 128:(mt + 1) * 128]
                        src = xt_ps[:, j * 128:(j + 1) * 128]
                        if (mt * 2 + j) % 4 != 3:
                            nc.vector.tensor_copy(out=dst, in_=src)
                        else:
                            nc.scalar.copy(out=dst, in_=src)

        # ---- P2: qkv projection (f32r) -> bf16 qT/kT/vT ----
        with tc.tile_pool(name="qkv_ps", bufs=4, space="PSUM") as qps:
            for ot in range(6):
                for ch in range(8):
                    acc = qps.tile([128, 512], F32, name="acc")
                    for j in range(2):
                        nc.tensor.matmul(
                            acc, wlhsT[j][:, ot, :],
                            xT[j][:, ch * 512:(ch + 1) * 512],
                            start=(j == 0), stop=(j == 1))
                    dst_j = ot % 2
                    if ot < 2:
                        dst = qT[dst_j][:, ch * 512:(ch + 1) * 512]
                    elif ot < 4:
                        dst = kT[dst_j][:, PAD + ch * 512:PAD + (ch + 1) * 512]
                    else:
                        dst = vT[dst_j][:, PAD + ch * 512:PAD + (ch + 1) * 512]
                    if (ot * 8 + ch) % 3 == 0:
                        nc.scalar.copy(out=dst, in_=acc)
                    else:
                        nc.vector.tensor_copy(out=dst, in_=acc)
        xt_pool.release()

        # ---- P3/P4, software-pipelined over m-chunks ----
        out_view = out_d[:, :].rearrange(
            "(s p) (j c) -> p s j c", p=128, j=2)

        with tc.tile_pool(name="prod_sb", bufs=1) as ttb, \
             tc.tile_pool(name="t2_sb", bufs=4) as t2b, \
             tc.tile_pool(name="bcd_sb", bufs=1) as bsb, \
             tc.tile_pool(name="sm_sb", bufs=4) as smb, \
             tc.tile_pool(name="sc_ps", bufs=2, space="PSUM") as sps, \
             tc.tile_pool(name="at_ps", bufs=2, space="PSUM") as aps, \
             tc.tile_pool(name="o_ps", bufs=1, space="PSUM") as ops, \
             tc.tile_pool(name="o_sb", bufs=2) as osb:

            def front(ch):
                """products -> scores -> softmax -> normalized at72 -> DRAM."""
                m0 = ch * MCH
                t_t = [[None] * K2 for _ in range(2)]
                for kk in range(K2):
                    dl = DELTAS[kk]
                    for j in range(2):
                        t = ttb.tile([128, MCH], BF16, name=f"pr{j}_{kk}")
                        if _prod_engine('qk', ch, j, kk) == 'pool':
                            nc.gpsimd.tensor_mul(
                                t, qT[j][:, m0:m0 + MCH],
                                kT[j][:, PAD + m0 + dl:PAD + m0 + MCH + dl])
                        else:
                            nc.vector.tensor_mul(
                                t, qT[j][:, m0:m0 + MCH],
                                kT[j][:, PAD + m0 + dl:PAD + m0 + MCH + dl])
                        t_t[j][kk] = t
                for g in range(SUBS // 4):
                    s_ps = sps.tile([128, 4, 72], F32, name="s_ps")
                    for sub4 in range(4):
                        sub = g * 4 + sub4
                        for kk in range(K2):
                            for j in range(2):
                                out_ap = s_ps.rearrange(
                                    "p s (h k) -> p s h k", k=9)[:, sub4, 4 * j:4 * j + 4, kk]
                                nc.tensor.matmul(
                                    out_ap,
                                    t_t[j][kk][:, sub * 128:sub * 128 + 128],
                                    ones_s, start=True, stop=True)
                    ms0 = ch * SUBS + g * 4
                    em0 = smb.tile([128, 4, 72], BF16, name="em0")
                    nc.scalar.activation(
                        em0, s_ps, mybir.ActivationFunctionType.Exp,
                        scale=float(SCALE))
                    em = smb.tile([128, 4, 72], BF16, name="em")
                    nc.vector.tensor_mul(em, em0, maskA[:, ms0:ms0 + 4, :])
                    den = smb.tile([128, 4, 8], F32, name="den")
                    nc.vector.reduce_sum(
                        den, em.rearrange("p s (h k) -> p s h k", k=9),
                        axis=mybir.AxisListType.X)
                    cb = cntT[:, ms0:ms0 + 4].unsqueeze(2).broadcast_to([128, 4, 8])
                    nc.vector.scalar_tensor_tensor(
                        out=den, in0=den, scalar=1.0, in1=cb,
                        op0=mybir.AluOpType.mult, op1=mybir.AluOpType.add)
                    rr = smb.tile([128, 4, 8], F32, name="rr")
                    nc.vector.reciprocal(rr, den)
                    rrb = smb.tile([128, 4, 8], BF16, name="rrb")
                    nc.vector.tensor_copy(out=rrb, in_=rr)
                    emn = smb.tile([128, 4, 72], BF16, name="emn")
                    rbc = rrb[:, :, :].unsqueeze(3).broadcast_to([128, 4, 8, 9])
                    nc.vector.scalar_tensor_tensor(
                        out=emn.rearrange("p s (h k) -> p s h k", k=9),
                        in0=em.rearrange("p s (h k) -> p s h k", k=9),
                        scalar=1.0, in1=rbc,
                        op0=mybir.AluOpType.mult, op1=mybir.AluOpType.mult)
                    at_ps = aps.tile([72, 4, 128], F32, name="at_ps")
                    for sub4 in range(4):
                        nc.tensor.matmul(
                            at_ps[:, sub4, :], emn[:, sub4, :], identb,
                            start=True, stop=True)
                    nc.scalar.copy(
                        out=at72[:, m0 + g * 512:m0 + (g + 1) * 512],
                        in_=at_ps.rearrange("p s q -> p (s q)"))
                nc.sync.dma_start(out=at_d[0:72, m0:m0 + MCH],
                                  in_=at72[:, m0:m0 + MCH])

            def back(ch):
                """DMA-broadcast attn, attn*v products, transpose-accumulate."""
                m0 = ch * MCH
                bc_t = [[None] * K2 for _ in range(2)]
                nd = 0
                for kk in range(K2):
                    for j in range(2):
                        bc = bsb.tile([128, MCH], BF16, name=f"bcd{j}_{kk}")
                        r0 = (4 * j) * 9 + kk
                        bap = at_d[r0:r0 + 28:9, m0:m0 + MCH]
                        bap = bap.unsqueeze(1).broadcast_to([4, 32, MCH])
                        qeng = [nc.sync, nc.gpsimd, nc.scalar][nd % 3]
                        nd += 1
                        qeng.dma_start(out=bc, in_=bap)
                        bc_t[j][kk] = (bc, None)
                o_sb = osb.tile([128, SUBS, 256], F32, name="o_sb")
                for half in range(2):
                    h0 = half * 512
                    o_gs = [ops.tile([128, 512], F32, name=f"o_g{sub4}")
                            for sub4 in range(4)]
                    for j in range(2):
                        for kk in range(K2):
                            dl = DELTAS[kk]
                            bch = bc_t[j][kk][0][:, h0:h0 + 512]
                            t2 = t2b.tile([128, 512], BF16, name=f"t2_{(j * K2 + kk) % 3}")
                            vsl = vT[j][:, PAD + m0 + h0 + dl:PAD + m0 + h0 + 512 + dl]
                            if _prod_engine('av', ch, j, kk) == 'pool':
                                nc.gpsimd.tensor_mul(t2, bch, vsl)
                            else:
                                nc.vector.tensor_mul(t2, bch, vsl)
                            for sub4 in range(4):
                                nc.tensor.matmul(
                                    o_gs[sub4][:, j * 128:(j + 1) * 128],
                                    t2[:, sub4 * 128:(sub4 + 1) * 128], identb,
                                    start=(kk == 0), stop=(kk == K2 - 1))
                    for sub4 in range(4):
                        sub = half * 4 + sub4
                        for j in range(2):
                            dst = o_sb[:, sub, j * 128:(j + 1) * 128]
                            src = o_gs[sub4][:, j * 128:(j + 1) * 128]
                            nc.scalar.copy(out=dst, in_=src)
                nc.sync.dma_start(
                    out=out_view[:, ch * SUBS:(ch + 1) * SUBS, :, :],
                    in_=o_sb.rearrange("p s (j c) -> p s j c", j=2))

            front(0)
            for ch in range(1, NCH):
                front(ch)
                back(ch - 1)
            back(NCH - 1)
    nc.compile()
    return nc


_NC_CACHE = None


def kernel(x: np.ndarray, W_qkv: np.ndarray) -> np.ndarray:
    global _NC_CACHE
    if _NC_CACHE is None:
        _NC_CACHE = build_nc()
    nc = _NC_CACHE

    x = np.ascontiguousarray(x, dtype=np.float32)
    W_qkv = np.ascontiguousarray(W_qkv, dtype=np.float32)
    ident, identb, ones_s, bkk, maskA, cntT = _host_consts()
    consts = {
        "w": W_qkv, "ident": ident, "identb": identb, "ones_s": ones_s,
        "bkk": bkk, "maskA": maskA, "cntT": cntT,
    }
    in_maps = [
        {"x": x[b].reshape(N, C).copy(), **consts} for b in range(B)
    ]
    res = run_bass_kernel_spmd(nc, in_maps, list(range(B)))
    out = np.stack([res.results[b]["out"].reshape(H, W, C) for b in range(B)])
    return out


if __name__ == "__main__":
    rng = np.random.default_rng(0)
    x = rng.standard_normal((B, H, W, C), dtype=np.float32)
    wq = (rng.standard_normal((3 * C, C), dtype=np.float32) * 0.02).astype(np.float32)
    out = kernel(x, wq)
    print("out", out.shape, out.dtype, float(np.abs(out).mean()))


# revision 8
# speedup vs baseline: 1.7583x; 1.0286x over previous
"""Trainium2 Bass kernel v2 for dilated local attention (nn_DilateAttention).

Problem: x [8, 64, 64, 256] f32, W_qkv [768, 256] f32.
  qkv = x @ W_qkv.T; per pixel, per head (8 heads x 32 dim): attention over
  the 9 dilated (3x3, dilation 3) spatial neighbors with zero padding.

Strategy (data-parallel over batch, 1 image per core), [c, m] on-chip layout:
  - PE: f32r transposes of x/W, f32r qkv projection, per-head score
    reduction with product-as-stationary matmuls, and AV accumulation via
    transpose-accumulate matmuls producing rows-layout output directly.
  - DVE/Pool: the q*k / attn*v elementwise products (bf16, SBUF-only so
    Pool is legal); DVE also runs the softmax chain batched 4 m-subs at a
    time on a single-bank [128, 4, 72] PSUM score tile.
  - Attention normalization (1/den) is applied in [m, 72] layout before
    transposing (free-broadcast scalar_tensor_tensor), then the normalized
    attention is transposed to [72, m] and round-tripped through a DRAM
    scratch so the per-channel broadcast becomes a partition-replicating
    (stride-0) DMA read - no PE/PSUM broadcast or evacuation needed.
  - Emission is software-pipelined: chunk ch+1's products/scores/softmax
    are emitted before chunk ch's AV phase so the in-order engine queues
    interleave the two chunks.
"""

import sys

sys.path.insert(0, "/opt/trn_rl_repo")

import numpy as np
import ml_dtypes
from contextlib import ExitStack

import concourse.bass as bass
import concourse.bacc as bacc
import concourse.tile as tile
from concourse import mybir
from concourse.bass_utils import run_bass_kernel_spmd

B, H, W, C = 8, 64, 64, 256
NH, DPH, K2 = 8, 32, 9
N = H * W          # 4096 pixels
PAD = 256          # zero border on each side of k/v (covers |delta| <= 195)
MCH = 1024         # pixels per m-chunk
NCH = N // MCH     # 4 chunks
SUBS = MCH // 128  # 8 m-subs per chunk
NSUB = N // 128    # 32 m-subs
SCALE = DPH ** -0.5
F32 = mybir.dt.float32
F32R = mybir.dt.float32r
BF16 = mybir.dt.bfloat16
NPBF16 = ml_dtypes.bfloat16

DELTAS = [64 * (3 * i - 3) + (3 * j - 3) for i in range(3) for j in range(3)]


def _prod_engine(phase, ch, j, kk):
    # Pool products are SBUF-only (HW-legal) and cheap in-model.
    if kk in (1, 4, 7):
        return 'pool'
    if kk == 0 and j == 1:
        return 'pool'
    return 'dve'


def _host_consts():
    ident = np.eye(128, dtype=np.float32)
    identb = np.eye(128, dtype=NPBF16)
    # score reduce (moving operand): ones_s[p, hh] = 1 iff p//32 == hh
    ones_s = np.zeros((128, 4), NPBF16)
    for p in range(128):
        ones_s[p, p // 32] = 1.0
    # kept for interface compat (unused when broadcast is DMA-only)
    bkk = np.zeros((72, 2, 9, 128), NPBF16)
    for jj in range(2):
        for kk in range(9):
            for q in range(128):
                bkk[(4 * jj + q // 32) * 9 + kk, jj, kk, q] = 1.0
    bkk = bkk.reshape(72, 2 * 9 * 128)
    # column-validity 0/1 mask in [m-sub, f=h*9+kk] layout, plus invalid
    # counts for the denominator (reference zero-pads keys: invalid slots
    # contribute exp(0)=1 to the denominator and 0 to the numerator).
    maskA = np.zeros((128, NSUB, 72), NPBF16)
    cntT = np.zeros((128, NSUB), np.float32)
    for ms in range(NSUB):
        m = ms * 128 + np.arange(128)
        jm = m % 64
        for kk in range(9):
            dc = 3 * (kk % 3) - 3
            valid = (((jm + dc) >= 0) & ((jm + dc) < 64)).astype(np.float32)
            for h in range(8):
                maskA[:, ms, h * 9 + kk] = valid
            cntT[:, ms] += 1.0 - valid
    maskA = maskA.reshape(128, NSUB * 72)
    return ident, identb, ones_s, bkk, maskA, cntT


def build_nc() -> bass.Bass:
    nc = bacc.Bacc()
    x_d = nc.declare_dram_parameter("x", [N, C], F32, isOutput=False)
    w_d = nc.declare_dram_parameter("w", [3 * C, C], F32, isOutput=False)
    ident_d = nc.declare_dram_parameter("ident", [128, 128], F32, isOutput=False)
    identb_d = nc.declare_dram_parameter("identb", [128, 128], BF16, isOutput=False)
    ones_s_d = nc.declare_dram_parameter("ones_s", [128, 4], BF16, isOutput=False)
    bkk_d = nc.declare_dram_parameter("bkk", [72, 2 * 9 * 128], BF16, isOutput=False)
    maskA_d = nc.declare_dram_parameter("maskA", [128, NSUB * 72], BF16, isOutput=False)
    cntT_d = nc.declare_dram_parameter("cntT", [128, NSUB], F32, isOutput=False)
    out_d = nc.declare_dram_parameter("out", [N, C], F32, isOutput=True)
    at_d = nc.dram_tensor("at_scratch", [80, N], BF16, kind="Internal")[:, :]

    with tile.TileContext(nc) as tc, ExitStack() as ctx:
        singles = ctx.enter_context(tc.tile_pool(name="singles", bufs=1))
        qkv_pool = ctx.enter_context(tc.tile_pool(name="qkv", bufs=1))

        identr = singles.tile([128, 128], F32R)
        nc.sync.dma_start(out=identr, in_=ident_d[:, :].bitcast(F32R))
        identb = singles.tile([128, 128], BF16)
        nc.sync.dma_start(out=identb, in_=identb_d[:, :])
        ones_s = singles.tile([128, 4], BF16)
        nc.gpsimd.dma_start(out=ones_s, in_=ones_s_d[:, :])
        maskA = singles.tile([128, NSUB, 72], BF16)
        nc.gpsimd.dma_start(
            out=maskA, in_=maskA_d[:, :].rearrange("p (s f) -> p s f", f=72))
        cntT = singles.tile([128, NSUB], F32)
        nc.gpsimd.dma_start(out=cntT, in_=cntT_d[:, :])

        # q/k/v in transposed [c, m] bf16 layout; k/v have zero borders of PAD
        qT = [qkv_pool.tile([128, N], BF16, name=f"qT{j}") for j in range(2)]
        kT = [qkv_pool.tile([128, N + 2 * PAD], BF16, name=f"kT{j}") for j in range(2)]
        vT = [qkv_pool.tile([128, N + 2 * PAD], BF16, name=f"vT{j}") for j in range(2)]
        for j in range(2):
            nc.gpsimd.memset(kT[j][:, 0:PAD], 0.0)
            nc.gpsimd.memset(kT[j][:, PAD + N:], 0.0)
            nc.gpsimd.memset(vT[j][:, 0:PAD], 0.0)
            nc.gpsimd.memset(vT[j][:, PAD + N:], 0.0)

        # normalized attention, [72 rows = h*9+kk, m]
        attn_pool = ctx.enter_context(tc.tile_pool(name="attn_sb", bufs=1))
        at72 = attn_pool.tile([72, N], BF16)

        # ---- P1: W^T tiles and x^T via PE transpose (f32r) ----
        xt_pool = tc.alloc_tile_pool(name="xt_pool", bufs=1)
        with tc.tile_pool(name="trans_sb", bufs=4) as tsb, \
             tc.tile_pool(name="trans_ps", bufs=2, space="PSUM") as tps:
            wlhsT = [singles.tile([128, 6, 128], F32R, name=f"wlhsT{j}") for j in range(2)]
            for ot in range(6):
                w_rows = tsb.tile([128, 256], F32R, name="w_rows")
                nc.sync.dma_start(out=w_rows, in_=w_d[ot * 128:(ot + 1) * 128, :].bitcast(F32R))
                for j in range(2):
                    wt_ps = tps.tile([128, 128], F32R, name="wt_ps")
                    nc.tensor.transpose(wt_ps, w_rows[:, j * 128:(j + 1) * 128], identr)
                    nc.scalar.copy(out=wlhsT[j][:, ot, :], in_=wt_ps)

            xT = [xt_pool.tile([128, N], F32R, name=f"xT{j}") for j in range(2)]
            xin = x_d[:, :].rearrange("(t p) c -> p t c", p=128).bitcast(F32R)
            for mb in range(8):
                x_rows = tsb.tile([128, 4, 256], F32R, name="x_rows")
                qeng = [nc.sync, nc.gpsimd, nc.scalar][mb % 3]
                qeng.dma_start(out=x_rows, in_=xin[:, mb * 4:(mb + 1) * 4, :])
                for t in range(4):
                    mt = mb * 4 + t
                    xt_ps = tps.tile([128, 256], F32R, name="xt_ps")
                    for j in range(2):
                        nc.tensor.transpose(
                            xt_ps[:, j * 128:(j + 1) * 128],
                            x_rows[:, t, j * 128:(j + 1) * 128], identr)
                    for j in range(2):
                        dst = xT[j][:, mt * 128:(mt + 1) * 128]
                        src = xt_ps[:, j * 128:(j + 1) * 128]
                        if (mt * 2 + j) % 4 != 3:
                            nc.vector.tensor_copy(out=dst, in_=src)
                        else:
                            nc.scalar.copy(out=dst, in_=src)

        # ---- P2: qkv projection (f32r) -> bf16 qT/kT/vT ----
        with tc.tile_pool(name="qkv_ps", bufs=4, space="PSUM") as qps:
            for ot in range(6):
                for ch in range(8):
                    acc = qps.tile([128, 512], F32, name="acc")
                    for j in range(2):
                        nc.tensor.matmul(
                            acc, wlhsT[j][:, ot, :],
                            xT[j][:, ch * 512:(ch + 1) * 512],
                            start=(j == 0), stop=(j == 1))
                    dst_j = ot % 2
                    if ot < 2:
                        dst = qT[dst_j][:, ch * 512:(ch + 1) * 512]
                    elif ot < 4:
                        dst = kT[dst_j][:, PAD + ch * 512:PAD + (ch + 1) * 512]
                    else:
                        dst = vT[dst_j][:, PAD + ch * 512:PAD + (ch + 1) * 512]
                    if (ot * 8 + ch) % 3 != 0:
                        nc.scalar.copy(out=dst, in_=acc)
                    else:
                        nc.vector.tensor_copy(out=dst, in_=acc)
        xt_pool.release()

        # ---- P3/P4, software-pipelined over m-chunks ----
        out_view = out_d[:, :].rearrange(
            "(s p) (j c) -> p s j c", p=128, j=2)

        with tc.tile_pool(name="prod_sb", bufs=1) as ttb, \
             tc.tile_pool(name="t2_sb", bufs=4) as t2b, \
             tc.tile_pool(name="bcd_sb", bufs=1) as bsb, \
             tc.tile_pool(name="sm_sb", bufs=4) as smb, \
             tc.tile_pool(name="sc_ps", bufs=2, space="PSUM") as sps, \
             tc.tile_pool(name="at_ps", bufs=2, space="PSUM") as aps, \
             tc.tile_pool(name="o_ps", bufs=1, space="PSUM") as ops, \
             tc.tile_pool(name="o_sb", bufs=2) as osb:

            def front(ch):
                """products -> scores -> softmax -> normalized at72 -> DRAM."""
                m0 = ch * MCH
                t_t = [[None] * K2 for _ in range(2)]
                for kk in range(K2):
                    dl = DELTAS[kk]
                    for j in range(2):
                        t = ttb.tile([128, MCH], BF16, name=f"pr{j}_{kk}")
                        if _prod_engine('qk', ch, j, kk) == 'pool':
                            nc.gpsimd.tensor_mul(
                                t, qT[j][:, m0:m0 + MCH],
                                kT[j][:, PAD + m0 + dl:PAD + m0 + MCH + dl])
                        else:
                            nc.vector.tensor_mul(
                                t, qT[j][:, m0:m0 + MCH],
                                kT[j][:, PAD + m0 + dl:PAD + m0 + MCH + dl])
                        t_t[j][kk] = t
                for g in range(SUBS // 4):
                    s_ps = sps.tile([128, 4, 72], F32, name="s_ps")
                    for sub4 in range(4):
                        sub = g * 4 + sub4
                        for kk in range(K2):
                            for j in range(2):
                                out_ap = s_ps.rearrange(
                                    "p s (h k) -> p s h k", k=9)[:, sub4, 4 * j:4 * j + 4, kk]
                                nc.tensor.matmul(
                                    out_ap,
                                    t_t[j][kk][:, sub * 128:sub * 128 + 128],
                                    ones_s, start=True, stop=True)
                    ms0 = ch * SUBS + g * 4
                    em0 = smb.tile([128, 4, 72], BF16, name="em0")
                    nc.scalar.activation(
                        em0, s_ps, mybir.ActivationFunctionType.Exp,
                        scale=float(SCALE))
                    em = smb.tile([128, 4, 72], BF16, name="em")
                    nc.vector.tensor_mul(em, em0, maskA[:, ms0:ms0 + 4, :])
                    den = smb.tile([128, 4, 8], F32, name="den")
                    nc.vector.reduce_sum(
                        den, em.rearrange("p s (h k) -> p s h k", k=9),
                        axis=mybir.AxisListType.X)
                    cb = cntT[:, ms0:ms0 + 4].unsqueeze(2).broadcast_to([128, 4, 8])
                    nc.vector.scalar_tensor_tensor(
                        out=den, in0=den, scalar=1.0, in1=cb,
                        op0=mybir.AluOpType.mult, op1=mybir.AluOpType.add)
                    rr = smb.tile([128, 4, 8], F32, name="rr")
                    nc.vector.reciprocal(rr, den)
                    rrb = smb.tile([128, 4, 8], BF16, name="rrb")
                    nc.vector.tensor_copy(out=rrb, in_=rr)
                    emn = smb.tile([128, 4, 72], BF16, name="emn")
                    rbc = rrb[:, :, :].unsqueeze(3).broadcast_to([128, 4, 8, 9])
                    nc.vector.scalar_tensor_tensor(
                        out=emn.rearrange("p s (h k) -> p s h k", k=9),
                        in0=em.rearrange("p s (h k) -> p s h k", k=9),
                        scalar=1.0, in1=rbc,
                        op0=mybir.AluOpType.mult, op1=mybir.AluOpType.mult)
                    at_ps = aps.tile([72, 4, 128], F32, name="at_ps")
                    for sub4 in range(4):
                        nc.tensor.matmul(
                            at_ps[:, sub4, :], emn[:, sub4, :], identb,
                            start=True, stop=True)
                    nc.scalar.copy(
                        out=at72[:, m0 + g * 512:m0 + (g + 1) * 512],
                        in_=at_ps.rearrange("p s q -> p (s q)"))
                nc.sync.dma_start(out=at_d[0:72, m0:m0 + MCH],
                                  in_=at72[:, m0:m0 + MCH])

            def back(ch):
                """DMA-broadcast attn, attn*v products, transpose-accumulate."""
                m0 = ch * MCH
                bc_t = [[None] * K2 for _ in range(2)]
                nd = 0
                for kk in range(K2):
                    for j in range(2):
                        bc = bsb.tile([128, MCH], BF16, name=f"bcd{j}_{kk}")
                        r0 = (4 * j) * 9 + kk
                        bap = at_d[r0:r0 + 28:9, m0:m0 + MCH]
                        bap = bap.unsqueeze(1).broadcast_to([4, 32, MCH])
                        qeng = [nc.sync, nc.gpsimd, nc.scalar][nd % 3]
                        nd += 1
                        qeng.dma_start(out=bc, in_=bap)
                        bc_t[j][kk] = (bc, None)
                o_sb = osb.tile([128, SUBS, 256], F32, name="o_sb")
                for half in range(2):
                    h0 = half * 512
                    o_gs = [ops.tile([128, 512], F32, name=f"o_g{sub4}")
                            for sub4 in range(4)]
                    for j in range(2):
                        for kk in range(K2):
                            dl = DELTAS[kk]
                            bch = bc_t[j][kk][0][:, h0:h0 + 512]
                            t2 = t2b.tile([128, 512], BF16, name=f"t2_{(j * K2 + kk) % 3}")
                            vsl = vT[j][:, PAD + m0 + h0 + dl:PAD + m0 + h0 + 512 + dl]
                            if _prod_engine('av', ch, j, kk) == 'pool':
                                nc.gpsimd.tensor_mul(t2, bch, vsl)
                            else:
                                nc.vector.tensor_mul(t2, bch, vsl)
                            for sub4 in range(4):
                                nc.tensor.matmul(
                                    o_gs[sub4][:, j * 128:(j + 1) * 128],
                                    t2[:, sub4 * 128:(sub4 + 1) * 128], identb,
                                    start=(kk == 0), stop=(kk == K2 - 1))
                    for sub4 in range(4):
                        sub = half * 4 + sub4
                        for j in range(2):
                            dst = o_sb[:, sub, j * 128:(j + 1) * 128]
                            src = o_gs[sub4][:, j * 128:(j + 1) * 128]
                            nc.scalar.copy(out=dst, in_=src)
                nc.sync.dma_start(
                    out=out_view[:, ch * SUBS:(ch + 1) * SUBS, :, :],
                    in_=o_sb.rearrange("p s (j c) -> p s j c", j=2))

            front(0)
            for ch in range(1, NCH):
                front(ch)
                back(ch - 1)
            back(NCH - 1)
    nc.compile()
    return nc


_NC_CACHE = None


def kernel(x: np.ndarray, W_qkv: np.ndarray) -> np.ndarray:
    global _NC_CACHE
    if _NC_CACHE is None:
        _NC_CACHE = build_nc()
    nc = _NC_CACHE

    x = np.ascontiguousarray(x, dtype=np.float32)
    W_qkv = np.ascontiguousarray(W_qkv, dtype=np.float32)
    ident, identb, ones_s, bkk, maskA, cntT = _host_consts()
    consts = {
        "w": W_qkv, "ident": ident, "identb": identb, "ones_s": ones_s,
        "bkk": bkk, "maskA": maskA, "cntT": cntT,
    }
    in_maps = [
        {"x": x[b].reshape(N, C).copy(), **consts} for b in range(B)
    ]
    res = run_bass_kernel_spmd(nc, in_maps, list(range(B)))
    out = np.stack([res.results[b]["out"].reshape(H, W, C) for b in range(B)])
    return out


if __name__ == "__main__":
    rng = np.random.default_rng(0)
    x = rng.standard_normal((B, H, W, C), dtype=np.float32)
    wq = (rng.standard_normal((3 * C, C), dtype=np.float32) * 0.02).astype(np.float32)
    out = kernel(x, wq)
    print("out", out.shape, out.dtype, float(np.abs(out).mean()))


# revision 9
# speedup vs baseline: 1.7911x; 1.0187x over previous
"""Trainium2 Bass kernel v2 for dilated local attention (nn_DilateAttention).

Problem: x [8, 64, 64, 256] f32, W_qkv [768, 256] f32.
  qkv = x @ W_qkv.T; per pixel, per head (8 heads x 32 dim): attention over
  the 9 dilated (3x3, dilation 3) spatial neighbors with zero padding.

Strategy (data-parallel over batch, 1 image per core), [c, m] on-chip layout:
  - PE: f32r transposes of x/W, f32r qkv projection, per-head score
    reduction with product-as-stationary matmuls, and AV accumulation via
    transpose-accumulate matmuls producing rows-layout output directly.
  - DVE/Pool: the q*k / attn*v elementwise products (bf16, SBUF-only so
    Pool is legal); DVE also runs the softmax chain batched 4 m-subs at a
    time on a single-bank [128, 4, 72] PSUM score tile.
  - Attention normalization (1/den) is applied in [m, 72] layout before
    transposing (free-broadcast scalar_tensor_tensor), then the normalized
    attention is transposed to [72, m] and round-tripped through a DRAM
    scratch so the per-channel broadcast becomes a partition-replicating
    (stride-0) DMA read - no PE/PSUM broadcast or evacuation needed.
  - Emission is software-pipelined: chunk ch+1's products/scores/softmax
    are emitted before chunk ch's AV phase so the in-order engine queues
    interleave the two chunks.
"""

import sys

sys.path.insert(0, "/opt/trn_rl_repo")

import numpy as np
import ml_dtypes
from contextlib import ExitStack

import concourse.bass as bass
import concourse.bacc as bacc
import concourse.tile as tile
from concourse import mybir
from concourse.bass_utils import run_bass_kernel_spmd

B, H, W, C = 8, 64, 64, 256
NH, DPH, K2 = 8, 32, 9
N = H * W          # 4096 pixels
PAD = 256          # zero border on each side of k/v (covers |delta| <= 195)
MCH = 1024         # pixels per m-chunk
NCH = N // MCH     # 4 chunks
SUBS = MCH // 128  # 8 m-subs per chunk
NSUB = N // 128    # 32 m-subs
SCALE = DPH ** -0.5
F32 = mybir.dt.float32
F32R = mybir.dt.float32r
BF16 = mybir.dt.bfloat16
NPBF16 = ml_dtypes.bfloat16

DELTAS = [64 * (3 * i - 3) + (3 * j - 3) for i in range(3) for j in range(3)]


def _prod_engine(phase, ch, j, kk):
    # Pool products are SBUF-only (HW-legal) and cheap in-model.
    if phase == 'qk':
        if kk in (1, 4, 7) or (kk in (0, 3) and j == 1):
            return 'pool'
        return 'dve'
    if kk in (1, 4, 7):
        return 'pool'
    return 'dve'


def _host_consts():
    ident = np.eye(128, dtype=np.float32)
    identb = np.eye(128, dtype=NPBF16)
    # score reduce (moving operand): ones_s[p, hh] = 1 iff p//32 == hh
    ones_s = np.zeros((128, 4), NPBF16)
    for p in range(128):
        ones_s[p, p // 32] = 1.0
    # kept for interface compat (unused when broadcast is DMA-only)
    bkk = np.zeros((72, 2, 9, 128), NPBF16)
    for jj in range(2):
        for kk in range(9):
            for q in range(128):
                bkk[(4 * jj + q // 32) * 9 + kk, jj, kk, q] = 1.0
    bkk = bkk.reshape(72, 2 * 9 * 128)
    # column-validity 0/1 mask in [m-sub, f=h*9+kk] layout, plus invalid
    # counts for the denominator (reference zero-pads keys: invalid slots
    # contribute exp(0)=1 to the denominator and 0 to the numerator).
    maskA = np.zeros((128, NSUB, 72), NPBF16)
    cntT = np.zeros((128, NSUB), np.float32)
    for ms in range(NSUB):
        m = ms * 128 + np.arange(128)
        jm = m % 64
        for kk in range(9):
            dc = 3 * (kk % 3) - 3
            valid = (((jm + dc) >= 0) & ((jm + dc) < 64)).astype(np.float32)
            for h in range(8):
                maskA[:, ms, h * 9 + kk] = valid
            cntT[:, ms] += 1.0 - valid
    maskA = maskA.reshape(128, NSUB * 72)
    return ident, identb, ones_s, bkk, maskA, cntT


def build_nc() -> bass.Bass:
    nc = bacc.Bacc()
    x_d = nc.declare_dram_parameter("x", [N, C], F32, isOutput=False)
    w_d = nc.declare_dram_parameter("w", [3 * C, C], F32, isOutput=False)
    ident_d = nc.declare_dram_parameter("ident", [128, 128], F32, isOutput=False)
    identb_d = nc.declare_dram_parameter("identb", [128, 128], BF16, isOutput=False)
    ones_s_d = nc.declare_dram_parameter("ones_s", [128, 4], BF16, isOutput=False)
    bkk_d = nc.declare_dram_parameter("bkk", [72, 2 * 9 * 128], BF16, isOutput=False)
    maskA_d = nc.declare_dram_parameter("maskA", [128, NSUB * 72], BF16, isOutput=False)
    cntT_d = nc.declare_dram_parameter("cntT", [128, NSUB], F32, isOutput=False)
    out_d = nc.declare_dram_parameter("out", [N, C], F32, isOutput=True)
    at_d = nc.dram_tensor("at_scratch", [80, N], BF16, kind="Internal")[:, :]

    with tile.TileContext(nc) as tc, ExitStack() as ctx:
        singles = ctx.enter_context(tc.tile_pool(name="singles", bufs=1))
        qkv_pool = ctx.enter_context(tc.tile_pool(name="qkv", bufs=1))

        identr = singles.tile([128, 128], F32R)
        nc.sync.dma_start(out=identr, in_=ident_d[:, :].bitcast(F32R))
        identb = singles.tile([128, 128], BF16)
        nc.sync.dma_start(out=identb, in_=identb_d[:, :])
        ones_s = singles.tile([128, 4], BF16)
        nc.gpsimd.dma_start(out=ones_s, in_=ones_s_d[:, :])
        maskA = singles.tile([128, NSUB, 72], BF16)
        nc.gpsimd.dma_start(
            out=maskA, in_=maskA_d[:, :].rearrange("p (s f) -> p s f", f=72))
        cntT = singles.tile([128, NSUB], F32)
        nc.gpsimd.dma_start(out=cntT, in_=cntT_d[:, :])

        # q/k/v in transposed [c, m] bf16 layout; k/v have zero borders of PAD
        qT = [qkv_pool.tile([128, N], BF16, name=f"qT{j}") for j in range(2)]
        kT = [qkv_pool.tile([128, N + 2 * PAD], BF16, name=f"kT{j}") for j in range(2)]
        vT = [qkv_pool.tile([128, N + 2 * PAD], BF16, name=f"vT{j}") for j in range(2)]
        for j in range(2):
            nc.gpsimd.memset(kT[j][:, 0:PAD], 0.0)
            nc.gpsimd.memset(kT[j][:, PAD + N:], 0.0)
            nc.gpsimd.memset(vT[j][:, 0:PAD], 0.0)
            nc.gpsimd.memset(vT[j][:, PAD + N:], 0.0)

        # normalized attention, [72 rows = h*9+kk, m]
        attn_pool = ctx.enter_context(tc.tile_pool(name="attn_sb", bufs=1))
        at72 = attn_pool.tile([72, N], BF16)

        # ---- P1: W^T tiles and x^T via PE transpose (f32r) ----
        xt_pool = tc.alloc_tile_pool(name="xt_pool", bufs=1)
        with tc.tile_pool(name="trans_sb", bufs=4) as tsb, \
             tc.tile_pool(name="trans_ps", bufs=2, space="PSUM") as tps:
            wlhsT = [singles.tile([128, 6, 128], F32R, name=f"wlhsT{j}") for j in range(2)]
            for ot in range(6):
                w_rows = tsb.tile([128, 256], F32R, name="w_rows")
                nc.sync.dma_start(out=w_rows, in_=w_d[ot * 128:(ot + 1) * 128, :].bitcast(F32R))
                for j in range(2):
                    wt_ps = tps.tile([128, 128], F32R, name="wt_ps")
                    nc.tensor.transpose(wt_ps, w_rows[:, j * 128:(j + 1) * 128], identr)
                    nc.scalar.copy(out=wlhsT[j][:, ot, :], in_=wt_ps)

            xT = [xt_pool.tile([128, N], F32R, name=f"xT{j}") for j in range(2)]
            xin = x_d[:, :].rearrange("(t p) c -> p t c", p=128).bitcast(F32R)
            for mb in range(8):
                x_rows = tsb.tile([128, 4, 256], F32R, name="x_rows")
                qeng = [nc.sync, nc.gpsimd, nc.scalar][mb % 3]
                qeng.dma_start(out=x_rows, in_=xin[:, mb * 4:(mb + 1) * 4, :])
                for t in range(4):
                    mt = mb * 4 + t
                    xt_ps = tps.tile([128, 256], F32R, name="xt_ps")
                    for j in range(2):
                        nc.tensor.transpose(
                            xt_ps[:, j * 128:(j + 1) * 128],
                            x_rows[:, t, j * 128:(j + 1) * 128], identr)
                    for j in range(2):
                        dst = xT[j][:, mt * 128:(mt + 1) * 128]
                        src = xt_ps[:, j * 128:(j + 1) * 128]
                        if (mt * 2 + j) % 4 != 3:
                            nc.vector.tensor_copy(out=dst, in_=src)
                        else:
                            nc.scalar.copy(out=dst, in_=src)

        # ---- P2: qkv projection (f32r) -> bf16 qT/kT/vT ----
        with tc.tile_pool(name="qkv_ps", bufs=4, space="PSUM") as qps:
            for ot in range(6):
                for ch in range(8):
                    acc = qps.tile([128, 512], F32, name="acc")
                    for j in range(2):
                        nc.tensor.matmul(
                            acc, wlhsT[j][:, ot, :],
                            xT[j][:, ch * 512:(ch + 1) * 512],
                            start=(j == 0), stop=(j == 1))
                    dst_j = ot % 2
                    if ot < 2:
                        dst = qT[dst_j][:, ch * 512:(ch + 1) * 512]
                    elif ot < 4:
                        dst = kT[dst_j][:, PAD + ch * 512:PAD + (ch + 1) * 512]
                    else:
                        dst = vT[dst_j][:, PAD + ch * 512:PAD + (ch + 1) * 512]
                    if (ot * 8 + ch) % 3 != 0:
                        nc.scalar.copy(out=dst, in_=acc)
                    else:
                        nc.vector.tensor_copy(out=dst, in_=acc)
        xt_pool.release()

        # ---- P3/P4, software-pipelined over m-chunks ----
        out_view = out_d[:, :].rearrange(
            "(s p) (j c) -> p s j c", p=128, j=2)

        with tc.tile_pool(name="prod_sb", bufs=1) as ttb, \
             tc.tile_pool(name="t2_sb", bufs=4) as t2b, \
             tc.tile_pool(name="bcd_sb", bufs=1) as bsb, \
             tc.tile_pool(name="sm_sb", bufs=4) as smb, \
             tc.tile_pool(name="sc_ps", bufs=2, space="PSUM") as sps, \
             tc.tile_pool(name="at_ps", bufs=2, space="PSUM") as aps, \
             tc.tile_pool(name="o_ps", bufs=1, space="PSUM") as ops, \
             tc.tile_pool(name="o_sb", bufs=2) as osb:

            def front(ch):
                """products -> scores -> softmax -> normalized at72 -> DRAM."""
                m0 = ch * MCH
                t_t = [[None] * K2 for _ in range(2)]
                for kk in range(K2):
                    dl = DELTAS[kk]
                    for j in range(2):
                        t = ttb.tile([128, MCH], BF16, name=f"pr{j}_{kk}")
                        if _prod_engine('qk', ch, j, kk) == 'pool':
                            nc.gpsimd.tensor_mul(
                                t, qT[j][:, m0:m0 + MCH],
                                kT[j][:, PAD + m0 + dl:PAD + m0 + MCH + dl])
                        else:
                            nc.vector.tensor_mul(
                                t, qT[j][:, m0:m0 + MCH],
                                kT[j][:, PAD + m0 + dl:PAD + m0 + MCH + dl])
                        t_t[j][kk] = t
                for g in range(SUBS // 4):
                    s_ps = sps.tile([128, 4, 72], F32, name="s_ps")
                    for sub4 in range(4):
                        sub = g * 4 + sub4
                        for kk in range(K2):
                            for j in range(2):
                                out_ap = s_ps.rearrange(
                                    "p s (h k) -> p s h k", k=9)[:, sub4, 4 * j:4 * j + 4, kk]
                                nc.tensor.matmul(
                                    out_ap,
                                    t_t[j][kk][:, sub * 128:sub * 128 + 128],
                                    ones_s, start=True, stop=True)
                    ms0 = ch * SUBS + g * 4
                    em0 = smb.tile([128, 4, 72], BF16, name="em0")
                    nc.scalar.activation(
                        em0, s_ps, mybir.ActivationFunctionType.Exp,
                        scale=float(SCALE))
                    em = smb.tile([128, 4, 72], BF16, name="em")
                    nc.vector.tensor_mul(em, em0, maskA[:, ms0:ms0 + 4, :])
                    den = smb.tile([128, 4, 8], F32, name="den")
                    nc.vector.reduce_sum(
                        den, em.rearrange("p s (h k) -> p s h k", k=9),
                        axis=mybir.AxisListType.X)
                    cb = cntT[:, ms0:ms0 + 4].unsqueeze(2).broadcast_to([128, 4, 8])
                    nc.vector.scalar_tensor_tensor(
                        out=den, in0=den, scalar=1.0, in1=cb,
                        op0=mybir.AluOpType.mult, op1=mybir.AluOpType.add)
                    rr = smb.tile([128, 4, 8], F32, name="rr")
                    nc.vector.reciprocal(rr, den)
                    rrb = smb.tile([128, 4, 8], BF16, name="rrb")
                    nc.vector.tensor_copy(out=rrb, in_=rr)
                    emn = smb.tile([128, 4, 72], BF16, name="emn")
                    rbc = rrb[:, :, :].unsqueeze(3).broadcast_to([128, 4, 8, 9])
                    nc.vector.scalar_tensor_tensor(
                        out=emn.rearrange("p s (h k) -> p s h k", k=9),
                        in0=em.rearrange("p s (h k) -> p s h k", k=9),
                        scalar=1.0, in1=rbc,
                        op0=mybir.AluOpType.mult, op1=mybir.AluOpType.mult)
                    at_ps = aps.tile([72, 4, 128], F32, name="at_ps")
                    for sub4 in range(4):
                        nc.tensor.matmul(
                            at_ps[:, sub4, :], emn[:, sub4, :], identb,
                            start=True, stop=True)
                    nc.scalar.copy(
                        out=at72[:, m0 + g * 512:m0 + (g + 1) * 512],
                        in_=at_ps.rearrange("p s q -> p (s q)"))
                nc.sync.dma_start(out=at_d[0:72, m0:m0 + MCH],
                                  in_=at72[:, m0:m0 + MCH])

            def back(ch):
                """DMA-broadcast attn, attn*v products, transpose-accumulate."""
                m0 = ch * MCH
                bc_t = [[None] * K2 for _ in range(2)]
                nd = 0
                for kk in range(K2):
                    for j in range(2):
                        bc = bsb.tile([128, MCH], BF16, name=f"bcd{j}_{kk}")
                        r0 = (4 * j) * 9 + kk
                        bap = at_d[r0:r0 + 28:9, m0:m0 + MCH]
                        bap = bap.unsqueeze(1).broadcast_to([4, 32, MCH])
                        qeng = [nc.sync, nc.gpsimd, nc.scalar][nd % 3]
                        nd += 1
                        qeng.dma_start(out=bc, in_=bap)
                        bc_t[j][kk] = (bc, None)
                o_sb = osb.tile([128, SUBS, 256], F32, name="o_sb")
                for half in range(2):
                    h0 = half * 512
                    o_gs = [ops.tile([128, 512], F32, name=f"o_g{sub4}")
                            for sub4 in range(4)]
                    for j in range(2):
                        for kk in range(K2):
                            dl = DELTAS[kk]
                            bch = bc_t[j][kk][0][:, h0:h0 + 512]
                            t2 = t2b.tile([128, 512], BF16, name=f"t2_{(j * K2 + kk) % 3}")
                            vsl = vT[j][:, PAD + m0 + h0 + dl:PAD + m0 + h0 + 512 + dl]
                            if _prod_engine('av', ch, j, kk) == 'pool':
                                nc.gpsimd.tensor_mul(t2, bch, vsl)
                            else:
                                nc.vector.tensor_mul(t2, bch, vsl)
                            for sub4 in range(4):
                                nc.tensor.matmul(
                                    o_gs[sub4][:, j * 128:(j + 1) * 128],
                                    t2[:, sub4 * 128:(sub4 + 1) * 128], identb,
                                    start=(kk == 0), stop=(kk == K2 - 1))
                    for sub4 in range(4):
                        sub = half * 4 + sub4
                        for j in range(2):
                            dst = o_sb[:, sub, j * 128:(j + 1) * 128]
                            src = o_gs[sub4][:, j * 128:(j + 1) * 128]
                            nc.scalar.copy(out=dst, in_=src)
                    s0 = ch * SUBS + half * 4
                    nc.sync.dma_start(
                        out=out_view[:, s0:s0 + 4, :, :],
                        in_=o_sb[:, half * 4:half * 4 + 4, :].rearrange(
                            "p s (j c) -> p s j c", j=2))

            front(0)
            for ch in range(1, NCH):
                front(ch)
                back(ch - 1)
            back(NCH - 1)
    nc.compile()
    return nc


_NC_CACHE = None


def kernel(x: np.ndarray, W_qkv: np.ndarray) -> np.ndarray:
    global _NC_CACHE
    if _NC_CACHE is None:
        _NC_CACHE = build_nc()
    nc = _NC_CACHE

    x = np.ascontiguousarray(x, dtype=np.float32)
    W_qkv = np.ascontiguousarray(W_qkv, dtype=np.float32)
    ident, identb, ones_s, bkk, maskA, cntT = _host_consts()
    consts = {
        "w": W_qkv, "ident": ident, "identb": identb, "ones_s": ones_s,
        "bkk": bkk, "maskA": maskA, "cntT": cntT,
    }
    in_maps = [
        {"x": x[b].reshape(N, C).copy(), **consts} for b in range(B)
    ]
    res = run_bass_kernel_spmd(nc, in_maps, list(range(B)))
    out = np.stack([res.results[b]["out"].reshape(H, W, C) for b in range(B)])
    return out


if __name__ == "__main__":
    rng = np.random.default_rng(0)
    x = rng.standard_normal((B, H, W, C), dtype=np.float32)
    wq = (rng.standard_normal((3 * C, C), dtype=np.float32) * 0.02).astype(np.float32)
    out = kernel(x, wq)
    print("out", out.shape, out.dtype, float(np.abs(out).mean()))


# revision 10
# speedup vs baseline: 1.8143x; 1.0129x over previous
"""Trainium2 Bass kernel v2 for dilated local attention (nn_DilateAttention).

Problem: x [8, 64, 64, 256] f32, W_qkv [768, 256] f32.
  qkv = x @ W_qkv.T; per pixel, per head (8 heads x 32 dim): attention over
  the 9 dilated (3x3, dilation 3) spatial neighbors with zero padding.

Strategy (data-parallel over batch, 1 image per core), [c, m] on-chip layout:
  - PE: f32r transposes of x/W, f32r qkv projection, per-head score
    reduction with product-as-stationary matmuls, and AV accumulation via
    transpose-accumulate matmuls producing rows-layout output directly.
  - DVE/Pool: the q*k / attn*v elementwise products (bf16, SBUF-only so
    Pool is legal); DVE also runs the softmax chain batched 4 m-subs at a
    time on a single-bank [128, 4, 72] PSUM score tile.
  - Attention normalization (1/den) is applied in [m, 72] layout before
    transposing (free-broadcast scalar_tensor_tensor), then the normalized
    attention is transposed to [72, m] and round-tripped through a DRAM
    scratch so the per-channel broadcast becomes a partition-replicating
    (stride-0) DMA read - no PE/PSUM broadcast or evacuation needed.
  - Emission is software-pipelined: chunk ch+1's products/scores/softmax
    are emitted before chunk ch's AV phase so the in-order engine queues
    interleave the two chunks.
"""

import sys

sys.path.insert(0, "/opt/trn_rl_repo")

import numpy as np
import ml_dtypes
from contextlib import ExitStack

import concourse.bass as bass
import concourse.bacc as bacc
import concourse.tile as tile
from concourse import mybir
from concourse.bass_utils import run_bass_kernel_spmd

B, H, W, C = 8, 64, 64, 256
NH, DPH, K2 = 8, 32, 9
N = H * W          # 4096 pixels
PAD = 256          # zero border on each side of k/v (covers |delta| <= 195)
MCH = 1024         # pixels per m-chunk
NCH = N // MCH     # 4 chunks
SUBS = MCH // 128  # 8 m-subs per chunk
NSUB = N // 128    # 32 m-subs
SCALE = DPH ** -0.5
F32 = mybir.dt.float32
F32R = mybir.dt.float32r
BF16 = mybir.dt.bfloat16
NPBF16 = ml_dtypes.bfloat16

DELTAS = [64 * (3 * i - 3) + (3 * j - 3) for i in range(3) for j in range(3)]


def _prod_engine(phase, ch, j, kk):
    # Pool products are SBUF-only (HW-legal) and cheap in-model.
    if phase == 'qk':
        if kk in (1, 4, 7) or (kk in (0, 3) and j == 1):
            return 'pool'
        return 'dve'
    if kk in (1, 4, 7):
        return 'pool'
    return 'dve'


def _host_consts():
    ident = np.eye(128, dtype=np.float32)
    identb = np.eye(128, dtype=NPBF16)
    # score reduce (moving operand): ones_s[p, hh] = 1 iff p//32 == hh
    ones_s = np.zeros((128, 4), NPBF16)
    for p in range(128):
        ones_s[p, p // 32] = 1.0
    # kept for interface compat (unused when broadcast is DMA-only)
    bkk = np.zeros((72, 2, 9, 128), NPBF16)
    for jj in range(2):
        for kk in range(9):
            for q in range(128):
                bkk[(4 * jj + q // 32) * 9 + kk, jj, kk, q] = 1.0
    bkk = bkk.reshape(72, 2 * 9 * 128)
    # column-validity 0/1 mask in [m-sub, f=h*9+kk] layout, plus invalid
    # counts for the denominator (reference zero-pads keys: invalid slots
    # contribute exp(0)=1 to the denominator and 0 to the numerator).
    maskA = np.zeros((128, NSUB, 72), NPBF16)
    cntT = np.zeros((128, NSUB), np.float32)
    for ms in range(NSUB):
        m = ms * 128 + np.arange(128)
        jm = m % 64
        for kk in range(9):
            dc = 3 * (kk % 3) - 3
            valid = (((jm + dc) >= 0) & ((jm + dc) < 64)).astype(np.float32)
            for h in range(8):
                maskA[:, ms, h * 9 + kk] = valid
            cntT[:, ms] += 1.0 - valid
    maskA = maskA.reshape(128, NSUB * 72)
    return ident, identb, ones_s, bkk, maskA, cntT


def build_nc() -> bass.Bass:
    nc = bacc.Bacc()
    x_d = nc.declare_dram_parameter("x", [N, C], F32, isOutput=False)
    w_d = nc.declare_dram_parameter("w", [3 * C, C], F32, isOutput=False)
    ident_d = nc.declare_dram_parameter("ident", [128, 128], F32, isOutput=False)
    identb_d = nc.declare_dram_parameter("identb", [128, 128], BF16, isOutput=False)
    ones_s_d = nc.declare_dram_parameter("ones_s", [128, 4], BF16, isOutput=False)
    bkk_d = nc.declare_dram_parameter("bkk", [72, 2 * 9 * 128], BF16, isOutput=False)
    maskA_d = nc.declare_dram_parameter("maskA", [128, NSUB * 72], BF16, isOutput=False)
    cntT_d = nc.declare_dram_parameter("cntT", [128, NSUB], F32, isOutput=False)
    out_d = nc.declare_dram_parameter("out", [N, C], F32, isOutput=True)
    at_d = nc.dram_tensor("at_scratch", [80, N], BF16, kind="Internal")[:, :]

    with tile.TileContext(nc) as tc, ExitStack() as ctx:
        singles = ctx.enter_context(tc.tile_pool(name="singles", bufs=1))
        qkv_pool = ctx.enter_context(tc.tile_pool(name="qkv", bufs=1))

        identr = singles.tile([128, 128], F32R)
        nc.sync.dma_start(out=identr, in_=ident_d[:, :].bitcast(F32R))
        identb = singles.tile([128, 128], BF16)
        nc.sync.dma_start(out=identb, in_=identb_d[:, :])
        ones_s = singles.tile([128, 4], BF16)
        nc.gpsimd.dma_start(out=ones_s, in_=ones_s_d[:, :])
        maskA = singles.tile([128, NSUB, 72], BF16)
        nc.gpsimd.dma_start(
            out=maskA, in_=maskA_d[:, :].rearrange("p (s f) -> p s f", f=72))
        cntT = singles.tile([128, NSUB], F32)
        nc.gpsimd.dma_start(out=cntT, in_=cntT_d[:, :])

        # q/k/v in transposed [c, m] bf16 layout; k/v have zero borders of PAD
        qT = [qkv_pool.tile([128, N], BF16, name=f"qT{j}") for j in range(2)]
        kT = [qkv_pool.tile([128, N + 2 * PAD], BF16, name=f"kT{j}") for j in range(2)]
        vT = [qkv_pool.tile([128, N + 2 * PAD], BF16, name=f"vT{j}") for j in range(2)]
        for j in range(2):
            nc.gpsimd.memset(kT[j][:, 0:PAD], 0.0)
            nc.gpsimd.memset(kT[j][:, PAD + N:], 0.0)
            nc.gpsimd.memset(vT[j][:, 0:PAD], 0.0)
            nc.gpsimd.memset(vT[j][:, PAD + N:], 0.0)

        # normalized attention, [72 rows = h*9+kk, m]
        attn_pool = ctx.enter_context(tc.tile_pool(name="attn_sb", bufs=1))
        at72 = attn_pool.tile([72, N], BF16)

        # ---- P1: W^T tiles and x^T via PE transpose (f32r) ----
        xt_pool = tc.alloc_tile_pool(name="xt_pool", bufs=1)
        with tc.tile_pool(name="trans_sb", bufs=4) as tsb, \
             tc.tile_pool(name="trans_ps", bufs=2, space="PSUM") as tps:
            wlhsT = [singles.tile([128, 6, 128], F32R, name=f"wlhsT{j}") for j in range(2)]
            for ot in range(6):
                w_rows = tsb.tile([128, 256], F32R, name="w_rows")
                nc.sync.dma_start(out=w_rows, in_=w_d[ot * 128:(ot + 1) * 128, :].bitcast(F32R))
                for j in range(2):
                    wt_ps = tps.tile([128, 128], F32R, name="wt_ps")
                    nc.tensor.transpose(wt_ps, w_rows[:, j * 128:(j + 1) * 128], identr)
                    nc.scalar.copy(out=wlhsT[j][:, ot, :], in_=wt_ps)

            xT = [xt_pool.tile([128, N], F32R, name=f"xT{j}") for j in range(2)]
            xin = x_d[:, :].rearrange("(t p) c -> p t c", p=128).bitcast(F32R)
            for mb in range(8):
                x_rows = tsb.tile([128, 4, 256], F32R, name="x_rows")
                qeng = [nc.sync, nc.gpsimd, nc.scalar][mb % 3]
                qeng.dma_start(out=x_rows, in_=xin[:, mb * 4:(mb + 1) * 4, :])
                for t in range(4):
                    mt = mb * 4 + t
                    xt_ps = tps.tile([128, 256], F32R, name="xt_ps")
                    for j in range(2):
                        nc.tensor.transpose(
                            xt_ps[:, j * 128:(j + 1) * 128],
                            x_rows[:, t, j * 128:(j + 1) * 128], identr)
                    for j in range(2):
                        dst = xT[j][:, mt * 128:(mt + 1) * 128]
                        src = xt_ps[:, j * 128:(j + 1) * 128]
                        if (mt * 2 + j) % 4 != 3:
                            nc.vector.tensor_copy(out=dst, in_=src)
                        else:
                            nc.scalar.copy(out=dst, in_=src)

        # ---- P2: qkv projection (f32r) -> bf16 qT/kT/vT ----
        with tc.tile_pool(name="qkv_ps", bufs=4, space="PSUM") as qps:
            for ot in range(6):
                for ch in range(8):
                    acc = qps.tile([128, 512], F32, name="acc")
                    for j in range(2):
                        nc.tensor.matmul(
                            acc, wlhsT[j][:, ot, :],
                            xT[j][:, ch * 512:(ch + 1) * 512],
                            start=(j == 0), stop=(j == 1))
                    dst_j = ot % 2
                    if ot < 2:
                        dst = qT[dst_j][:, ch * 512:(ch + 1) * 512]
                    elif ot < 4:
                        dst = kT[dst_j][:, PAD + ch * 512:PAD + (ch + 1) * 512]
                    else:
                        dst = vT[dst_j][:, PAD + ch * 512:PAD + (ch + 1) * 512]
                    if (ot * 8 + ch) % 3 != 0:
                        nc.scalar.copy(out=dst, in_=acc)
                    else:
                        nc.vector.tensor_copy(out=dst, in_=acc)
        xt_pool.release()

        # ---- P3/P4, software-pipelined over m-chunks ----
        out_view = out_d[:, :].rearrange(
            "(s p) (j c) -> p s j c", p=128, j=2)

        with tc.tile_pool(name="prod_sb", bufs=1) as ttb, \
             tc.tile_pool(name="t2_sb", bufs=4) as t2b, \
             tc.tile_pool(name="bcd_sb", bufs=1) as bsb, \
             tc.tile_pool(name="sm_sb", bufs=4) as smb, \
             tc.tile_pool(name="sc_ps", bufs=2, space="PSUM") as sps, \
             tc.tile_pool(name="at_ps", bufs=2, space="PSUM") as aps, \
             tc.tile_pool(name="o_ps", bufs=1, space="PSUM") as ops, \
             tc.tile_pool(name="o_sb", bufs=2) as osb:

            def front(ch):
                """products -> scores -> softmax -> normalized at72 -> DRAM."""
                m0 = ch * MCH
                t_t = [[None] * K2 for _ in range(2)]
                for kk in range(K2):
                    dl = DELTAS[kk]
                    for j in range(2):
                        t = ttb.tile([128, MCH], BF16, name=f"pr{j}_{kk}")
                        if _prod_engine('qk', ch, j, kk) == 'pool':
                            nc.gpsimd.tensor_mul(
                                t, qT[j][:, m0:m0 + MCH],
                                kT[j][:, PAD + m0 + dl:PAD + m0 + MCH + dl])
                        else:
                            nc.vector.tensor_mul(
                                t, qT[j][:, m0:m0 + MCH],
                                kT[j][:, PAD + m0 + dl:PAD + m0 + MCH + dl])
                        t_t[j][kk] = t
                for g in range(SUBS // 4):
                    s_ps = sps.tile([128, 4, 72], F32, name="s_ps")
                    for sub4 in range(4):
                        sub = g * 4 + sub4
                        for kk in range(K2):
                            for j in range(2):
                                out_ap = s_ps.rearrange(
                                    "p s (h k) -> p s h k", k=9)[:, sub4, 4 * j:4 * j + 4, kk]
                                nc.tensor.matmul(
                                    out_ap,
                                    t_t[j][kk][:, sub * 128:sub * 128 + 128],
                                    ones_s, start=True, stop=True)
                    ms0 = ch * SUBS + g * 4
                    em0 = smb.tile([128, 4, 72], BF16, name="em0")
                    nc.scalar.activation(
                        em0, s_ps, mybir.ActivationFunctionType.Exp,
                        scale=float(SCALE))
                    em = smb.tile([128, 4, 72], BF16, name="em")
                    nc.vector.tensor_mul(em, em0, maskA[:, ms0:ms0 + 4, :])
                    den = smb.tile([128, 4, 8], F32, name="den")
                    nc.vector.reduce_sum(
                        den, em.rearrange("p s (h k) -> p s h k", k=9),
                        axis=mybir.AxisListType.X)
                    cb = cntT[:, ms0:ms0 + 4].unsqueeze(2).broadcast_to([128, 4, 8])
                    nc.vector.scalar_tensor_tensor(
                        out=den, in0=den, scalar=1.0, in1=cb,
                        op0=mybir.AluOpType.mult, op1=mybir.AluOpType.add)
                    rr = smb.tile([128, 4, 8], F32, name="rr")
                    nc.vector.reciprocal(rr, den)
                    rrb = smb.tile([128, 4, 8], BF16, name="rrb")
                    nc.vector.tensor_copy(out=rrb, in_=rr)
                    emn = smb.tile([128, 4, 72], BF16, name="emn")
                    rbc = rrb[:, :, :].unsqueeze(3).broadcast_to([128, 4, 8, 9])
                    nc.vector.scalar_tensor_tensor(
                        out=emn.rearrange("p s (h k) -> p s h k", k=9),
                        in0=em.rearrange("p s (h k) -> p s h k", k=9),
                        scalar=1.0, in1=rbc,
                        op0=mybir.AluOpType.mult, op1=mybir.AluOpType.mult)
                    at_ps = aps.tile([72, 4, 128], F32, name="at_ps")
                    for sub4 in range(4):
                        nc.tensor.matmul(
                            at_ps[:, sub4, :], emn[:, sub4, :], identb,
                            start=True, stop=True)
                    nc.scalar.copy(
                        out=at72[:, m0 + g * 512:m0 + (g + 1) * 512],
                        in_=at_ps.rearrange("p s q -> p (s q)"))
                nc.sync.dma_start(out=at_d[0:72, m0:m0 + MCH],
                                  in_=at72[:, m0:m0 + MCH])

            def back(ch):
                """DMA-broadcast attn, attn*v products, transpose-accumulate."""
                m0 = ch * MCH
                bc_t = [[None] * K2 for _ in range(2)]
                nd = 0
                for kk in range(K2):
                    for j in range(2):
                        bc = bsb.tile([128, MCH], BF16, name=f"bcd{j}_{kk}")
                        r0 = (4 * j) * 9 + kk
                        bap = at_d[r0:r0 + 28:9, m0:m0 + MCH]
                        bap = bap.unsqueeze(1).broadcast_to([4, 32, MCH])
                        qeng = [nc.sync, nc.gpsimd, nc.scalar][nd % 3]
                        nd += 1
                        qeng.dma_start(out=bc, in_=bap)
                        bc_t[j][kk] = (bc, None)
                o_sb = osb.tile([128, SUBS, 256], F32, name="o_sb")
                for half in range(2):
                    h0 = half * 512
                    o_gs = [ops.tile([128, 512], F32, name=f"o_g{sub4}")
                            for sub4 in range(4)]
                    for j in range(2):
                        for kk in range(K2):
                            dl = DELTAS[kk]
                            bch = bc_t[j][kk][0][:, h0:h0 + 512]
                            t2 = t2b.tile([128, 512], BF16, name=f"t2_{(j * K2 + kk) % 3}")
                            vsl = vT[j][:, PAD + m0 + h0 + dl:PAD + m0 + h0 + 512 + dl]
                            if _prod_engine('av', ch, j, kk) == 'pool':
                                nc.gpsimd.tensor_mul(t2, bch, vsl)
                            else:
                                nc.vector.tensor_mul(t2, bch, vsl)
                            for sub4 in range(4):
                                nc.tensor.matmul(
                                    o_gs[sub4][:, j * 128:(j + 1) * 128],
                                    t2[:, sub4 * 128:(sub4 + 1) * 128], identb,
                                    start=(kk == 0), stop=(kk == K2 - 1))
                        # evacuate this j's closed groups while the other j runs
                        for sub4 in range(4):
                            sub = half * 4 + sub4
                            dst = o_sb[:, sub, j * 128:(j + 1) * 128]
                            src = o_gs[sub4][:, j * 128:(j + 1) * 128]
                            nc.scalar.copy(out=dst, in_=src)
                    s0 = ch * SUBS + half * 4
                    nc.sync.dma_start(
                        out=out_view[:, s0:s0 + 4, :, :],
                        in_=o_sb[:, half * 4:half * 4 + 4, :].rearrange(
                            "p s (j c) -> p s j c", j=2))

            front(0)
            for ch in range(1, NCH):
                front(ch)
                back(ch - 1)
            back(NCH - 1)
    nc.compile()
    return nc


_NC_CACHE = None


def kernel(x: np.ndarray, W_qkv: np.ndarray) -> np.ndarray:
    global _NC_CACHE
    if _NC_CACHE is None:
        _NC_CACHE = build_nc()
    nc = _NC_CACHE

    x = np.ascontiguousarray(x, dtype=np.float32)
    W_qkv = np.ascontiguousarray(W_qkv, dtype=np.float32)
    ident, identb, ones_s, bkk, maskA, cntT = _host_consts()
    consts = {
        "w": W_qkv, "ident": ident, "identb": identb, "ones_s": ones_s,
        "bkk": bkk, "maskA": maskA, "cntT": cntT,
    }
    in_maps = [
        {"x": x[b].reshape(N, C).copy(), **consts} for b in range(B)
    ]
    res = run_bass_kernel_spmd(nc, in_maps, list(range(B)))
    out = np.stack([res.results[b]["out"].reshape(H, W, C) for b in range(B)])
    return out


if __name__ == "__main__":
    rng = np.random.default_rng(0)
    x = rng.standard_normal((B, H, W, C), dtype=np.float32)
    wq = (rng.standard_normal((3 * C, C), dtype=np.float32) * 0.02).astype(np.float32)
    out = kernel(x, wq)
    print("out", out.shape, out.dtype, float(np.abs(out).mean()))


# revision 11
# speedup vs baseline: 1.8166x; 1.0013x over previous
"""Trainium2 Bass kernel v2 for dilated local attention (nn_DilateAttention).

Problem: x [8, 64, 64, 256] f32, W_qkv [768, 256] f32.
  qkv = x @ W_qkv.T; per pixel, per head (8 heads x 32 dim): attention over
  the 9 dilated (3x3, dilation 3) spatial neighbors with zero padding.

Strategy (data-parallel over batch, 1 image per core), [c, m] on-chip layout:
  - PE: f32r transposes of x/W, f32r qkv projection, per-head score
    reduction with product-as-stationary matmuls, and AV accumulation via
    transpose-accumulate matmuls producing rows-layout output directly.
  - DVE/Pool: the q*k / attn*v elementwise products (bf16, SBUF-only so
    Pool is legal); DVE also runs the softmax chain batched 4 m-subs at a
    time on a single-bank [128, 4, 72] PSUM score tile.
  - Attention normalization (1/den) is applied in [m, 72] layout before
    transposing (free-broadcast scalar_tensor_tensor), then the normalized
    attention is transposed to [72, m] and round-tripped through a DRAM
    scratch so the per-channel broadcast becomes a partition-replicating
    (stride-0) DMA read - no PE/PSUM broadcast or evacuation needed.
  - Emission is software-pipelined: chunk ch+1's products/scores/softmax
    are emitted before chunk ch's AV phase so the in-order engine queues
    interleave the two chunks.
"""

import sys

sys.path.insert(0, "/opt/trn_rl_repo")

import numpy as np
import ml_dtypes
from contextlib import ExitStack

import concourse.bass as bass
import concourse.bacc as bacc
import concourse.tile as tile
from concourse import mybir
from concourse.bass_utils import run_bass_kernel_spmd

B, H, W, C = 8, 64, 64, 256
NH, DPH, K2 = 8, 32, 9
N = H * W          # 4096 pixels
PAD = 256          # zero border on each side of k/v (covers |delta| <= 195)
MCH = 1024         # pixels per m-chunk
NCH = N // MCH     # 4 chunks
SUBS = MCH // 128  # 8 m-subs per chunk
NSUB = N // 128    # 32 m-subs
SCALE = DPH ** -0.5
F32 = mybir.dt.float32
F32R = mybir.dt.float32r
BF16 = mybir.dt.bfloat16
NPBF16 = ml_dtypes.bfloat16

DELTAS = [64 * (3 * i - 3) + (3 * j - 3) for i in range(3) for j in range(3)]


def _prod_engine(phase, ch, j, kk):
    # Pool products are SBUF-only (HW-legal) and cheap in-model.
    if phase == 'qk':
        if kk in (1, 4, 7) or (kk in (0, 3) and j == 1):
            return 'pool'
        return 'dve'
    if kk in (1, 4, 7):
        return 'pool'
    return 'dve'


def _host_consts():
    ident = np.eye(128, dtype=np.float32)
    identb = np.eye(128, dtype=NPBF16)
    # score reduce (moving operand): ones_s[p, hh] = 1 iff p//32 == hh
    ones_s = np.zeros((128, 4), NPBF16)
    for p in range(128):
        ones_s[p, p // 32] = 1.0
    # kept for interface compat (unused when broadcast is DMA-only)
    bkk = np.zeros((72, 2, 9, 128), NPBF16)
    for jj in range(2):
        for kk in range(9):
            for q in range(128):
                bkk[(4 * jj + q // 32) * 9 + kk, jj, kk, q] = 1.0
    bkk = bkk.reshape(72, 2 * 9 * 128)
    # column-validity 0/1 mask in [m-sub, f=h*9+kk] layout, plus invalid
    # counts for the denominator (reference zero-pads keys: invalid slots
    # contribute exp(0)=1 to the denominator and 0 to the numerator).
    maskA = np.zeros((128, NSUB, 72), NPBF16)
    cntT = np.zeros((128, NSUB), np.float32)
    for ms in range(NSUB):
        m = ms * 128 + np.arange(128)
        jm = m % 64
        for kk in range(9):
            dc = 3 * (kk % 3) - 3
            valid = (((jm + dc) >= 0) & ((jm + dc) < 64)).astype(np.float32)
            for h in range(8):
                maskA[:, ms, h * 9 + kk] = valid
            cntT[:, ms] += 1.0 - valid
    maskA = maskA.reshape(128, NSUB * 72)
    return ident, identb, ones_s, bkk, maskA, cntT


def build_nc() -> bass.Bass:
    nc = bacc.Bacc()
    x_d = nc.declare_dram_parameter("x", [N, C], F32, isOutput=False)
    w_d = nc.declare_dram_parameter("w", [3 * C, C], F32, isOutput=False)
    ident_d = nc.declare_dram_parameter("ident", [128, 128], F32, isOutput=False)
    identb_d = nc.declare_dram_parameter("identb", [128, 128], BF16, isOutput=False)
    ones_s_d = nc.declare_dram_parameter("ones_s", [128, 4], BF16, isOutput=False)
    bkk_d = nc.declare_dram_parameter("bkk", [72, 2 * 9 * 128], BF16, isOutput=False)
    maskA_d = nc.declare_dram_parameter("maskA", [128, NSUB * 72], BF16, isOutput=False)
    cntT_d = nc.declare_dram_parameter("cntT", [128, NSUB], F32, isOutput=False)
    out_d = nc.declare_dram_parameter("out", [N, C], F32, isOutput=True)
    at_d = nc.dram_tensor("at_scratch", [80, N], BF16, kind="Internal")[:, :]

    with tile.TileContext(nc) as tc, ExitStack() as ctx:
        singles = ctx.enter_context(tc.tile_pool(name="singles", bufs=1))
        qkv_pool = ctx.enter_context(tc.tile_pool(name="qkv", bufs=1))

        identr = singles.tile([128, 128], F32R)
        nc.sync.dma_start(out=identr, in_=ident_d[:, :].bitcast(F32R))
        identb = singles.tile([128, 128], BF16)
        nc.sync.dma_start(out=identb, in_=identb_d[:, :])
        ones_s = singles.tile([128, 4], BF16)
        nc.gpsimd.dma_start(out=ones_s, in_=ones_s_d[:, :])
        maskA = singles.tile([128, NSUB, 72], BF16)
        nc.gpsimd.dma_start(
            out=maskA, in_=maskA_d[:, :].rearrange("p (s f) -> p s f", f=72))
        cntT = singles.tile([128, NSUB], F32)
        nc.gpsimd.dma_start(out=cntT, in_=cntT_d[:, :])

        # q/k/v in transposed [c, m] bf16 layout; k/v have zero borders of PAD
        qT = [qkv_pool.tile([128, N], BF16, name=f"qT{j}") for j in range(2)]
        kT = [qkv_pool.tile([128, N + 2 * PAD], BF16, name=f"kT{j}") for j in range(2)]
        vT = [qkv_pool.tile([128, N + 2 * PAD], BF16, name=f"vT{j}") for j in range(2)]
        for j in range(2):
            nc.gpsimd.memset(kT[j][:, 0:PAD], 0.0)
            nc.gpsimd.memset(kT[j][:, PAD + N:], 0.0)
            nc.gpsimd.memset(vT[j][:, 0:PAD], 0.0)
            nc.gpsimd.memset(vT[j][:, PAD + N:], 0.0)

        # normalized attention, [72 rows = h*9+kk, m]
        attn_pool = ctx.enter_context(tc.tile_pool(name="attn_sb", bufs=1))
        at72 = attn_pool.tile([72, N], BF16)

        # ---- P1: W^T tiles and x^T via PE transpose (f32r) ----
        xt_pool = tc.alloc_tile_pool(name="xt_pool", bufs=1)
        with tc.tile_pool(name="trans_sb", bufs=4) as tsb, \
             tc.tile_pool(name="trans_ps", bufs=2, space="PSUM") as tps:
            wlhsT = [singles.tile([128, 6, 128], F32R, name=f"wlhsT{j}") for j in range(2)]
            for ot in range(6):
                w_rows = tsb.tile([128, 256], F32R, name="w_rows")
                nc.sync.dma_start(out=w_rows, in_=w_d[ot * 128:(ot + 1) * 128, :].bitcast(F32R))
                for j in range(2):
                    wt_ps = tps.tile([128, 128], F32R, name="wt_ps")
                    nc.tensor.transpose(wt_ps, w_rows[:, j * 128:(j + 1) * 128], identr)
                    nc.scalar.copy(out=wlhsT[j][:, ot, :], in_=wt_ps)

            xT = [xt_pool.tile([128, N], F32R, name=f"xT{j}") for j in range(2)]
            xin = x_d[:, :].rearrange("(t p) c -> p t c", p=128).bitcast(F32R)
            for mb in range(8):
                x_rows = tsb.tile([128, 4, 256], F32R, name="x_rows")
                qeng = [nc.sync, nc.gpsimd, nc.scalar][mb % 3]
                qeng.dma_start(out=x_rows, in_=xin[:, mb * 4:(mb + 1) * 4, :])
                for t in range(4):
                    mt = mb * 4 + t
                    xt_ps = tps.tile([128, 256], F32R, name="xt_ps")
                    for j in range(2):
                        nc.tensor.transpose(
                            xt_ps[:, j * 128:(j + 1) * 128],
                            x_rows[:, t, j * 128:(j + 1) * 128], identr)
                    for j in range(2):
                        dst = xT[j][:, mt * 128:(mt + 1) * 128]
                        src = xt_ps[:, j * 128:(j + 1) * 128]
                        if (mt * 2 + j) % 4 != 3:
                            nc.vector.tensor_copy(out=dst, in_=src)
                        else:
                            nc.scalar.copy(out=dst, in_=src)

        # ---- P2: qkv projection (f32r) -> bf16 qT/kT/vT ----
        with tc.tile_pool(name="qkv_ps", bufs=4, space="PSUM") as qps:
            for ot in range(6):
                for ch in range(8):
                    acc = qps.tile([128, 512], F32, name="acc")
                    for j in range(2):
                        nc.tensor.matmul(
                            acc, wlhsT[j][:, ot, :],
                            xT[j][:, ch * 512:(ch + 1) * 512],
                            start=(j == 0), stop=(j == 1))
                    dst_j = ot % 2
                    if ot < 2:
                        dst = qT[dst_j][:, ch * 512:(ch + 1) * 512]
                    elif ot < 4:
                        dst = kT[dst_j][:, PAD + ch * 512:PAD + (ch + 1) * 512]
                    else:
                        dst = vT[dst_j][:, PAD + ch * 512:PAD + (ch + 1) * 512]
                    if (ot * 8 + ch) % 3 != 0:
                        nc.scalar.copy(out=dst, in_=acc)
                    else:
                        nc.vector.tensor_copy(out=dst, in_=acc)
        xt_pool.release()

        # ---- P3/P4, software-pipelined over m-chunks ----
        out_view = out_d[:, :].rearrange(
            "(s p) (j c) -> p s j c", p=128, j=2)

        with tc.tile_pool(name="prod_sb", bufs=1) as ttb, \
             tc.tile_pool(name="t2_sb", bufs=4) as t2b, \
             tc.tile_pool(name="bcd_sb", bufs=1) as bsb, \
             tc.tile_pool(name="sm_sb", bufs=4) as smb, \
             tc.tile_pool(name="sc_ps", bufs=2, space="PSUM") as sps, \
             tc.tile_pool(name="at_ps", bufs=2, space="PSUM") as aps, \
             tc.tile_pool(name="o_ps", bufs=1, space="PSUM") as ops, \
             tc.tile_pool(name="o_sb", bufs=2) as osb:

            def front(ch):
                """products -> scores -> softmax -> normalized at72 -> DRAM."""
                m0 = ch * MCH
                t_t = [[None] * K2 for _ in range(2)]
                for kk in range(K2):
                    dl = DELTAS[kk]
                    for j in range(2):
                        t = ttb.tile([128, MCH], BF16, name=f"pr{j}_{kk}")
                        if _prod_engine('qk', ch, j, kk) == 'pool':
                            nc.gpsimd.tensor_mul(
                                t, qT[j][:, m0:m0 + MCH],
                                kT[j][:, PAD + m0 + dl:PAD + m0 + MCH + dl])
                        else:
                            nc.vector.tensor_mul(
                                t, qT[j][:, m0:m0 + MCH],
                                kT[j][:, PAD + m0 + dl:PAD + m0 + MCH + dl])
                        t_t[j][kk] = t
                for g in range(SUBS // 4):
                    s_ps = sps.tile([128, 4, 72], F32, name="s_ps")
                    for sub4 in range(4):
                        sub = g * 4 + sub4
                        for kk in range(K2):
                            for j in range(2):
                                out_ap = s_ps.rearrange(
                                    "p s (h k) -> p s h k", k=9)[:, sub4, 4 * j:4 * j + 4, kk]
                                nc.tensor.matmul(
                                    out_ap,
                                    t_t[j][kk][:, sub * 128:sub * 128 + 128],
                                    ones_s, start=True, stop=True)
                    ms0 = ch * SUBS + g * 4
                    em0 = smb.tile([128, 4, 72], BF16, name="em0")
                    nc.scalar.activation(
                        em0, s_ps, mybir.ActivationFunctionType.Exp,
                        scale=float(SCALE))
                    em = smb.tile([128, 4, 72], BF16, name="em")
                    nc.vector.tensor_mul(em, em0, maskA[:, ms0:ms0 + 4, :])
                    den = smb.tile([128, 4, 8], F32, name="den")
                    nc.vector.reduce_sum(
                        den, em.rearrange("p s (h k) -> p s h k", k=9),
                        axis=mybir.AxisListType.X)
                    cb = cntT[:, ms0:ms0 + 4].unsqueeze(2).broadcast_to([128, 4, 8])
                    nc.vector.scalar_tensor_tensor(
                        out=den, in0=den, scalar=1.0, in1=cb,
                        op0=mybir.AluOpType.mult, op1=mybir.AluOpType.add)
                    rr = smb.tile([128, 4, 8], F32, name="rr")
                    nc.vector.reciprocal(rr, den)
                    emn = smb.tile([128, 4, 72], BF16, name="emn")
                    rbc = rr[:, :, :].unsqueeze(3).broadcast_to([128, 4, 8, 9])
                    nc.vector.scalar_tensor_tensor(
                        out=emn.rearrange("p s (h k) -> p s h k", k=9),
                        in0=em.rearrange("p s (h k) -> p s h k", k=9),
                        scalar=1.0, in1=rbc,
                        op0=mybir.AluOpType.mult, op1=mybir.AluOpType.mult)
                    at_ps = aps.tile([72, 4, 128], F32, name="at_ps")
                    for sub4 in range(4):
                        nc.tensor.matmul(
                            at_ps[:, sub4, :], emn[:, sub4, :], identb,
                            start=True, stop=True)
                    nc.scalar.copy(
                        out=at72[:, m0 + g * 512:m0 + (g + 1) * 512],
                        in_=at_ps.rearrange("p s q -> p (s q)"))
                nc.sync.dma_start(out=at_d[0:72, m0:m0 + MCH],
                                  in_=at72[:, m0:m0 + MCH])

            def back(ch):
                """DMA-broadcast attn, attn*v products, transpose-accumulate."""
                m0 = ch * MCH
                bc_t = [[None] * K2 for _ in range(2)]
                nd = 0
                for kk in range(K2):
                    for j in range(2):
                        bc = bsb.tile([128, MCH], BF16, name=f"bcd{j}_{kk}")
                        r0 = (4 * j) * 9 + kk
                        bap = at_d[r0:r0 + 28:9, m0:m0 + MCH]
                        bap = bap.unsqueeze(1).broadcast_to([4, 32, MCH])
                        qeng = [nc.sync, nc.gpsimd, nc.scalar][nd % 3]
                        nd += 1
                        qeng.dma_start(out=bc, in_=bap)
                        bc_t[j][kk] = (bc, None)
                o_sb = osb.tile([128, SUBS, 256], F32, name="o_sb")
                for half in range(2):
                    h0 = half * 512
                    o_gs = [ops.tile([128, 512], F32, name=f"o_g{sub4}")
                            for sub4 in range(4)]
                    for j in range(2):
                        for kk in range(K2):
                            dl = DELTAS[kk]
                            bch = bc_t[j][kk][0][:, h0:h0 + 512]
                            t2 = t2b.tile([128, 512], BF16, name=f"t2_{(j * K2 + kk) % 3}")
                            vsl = vT[j][:, PAD + m0 + h0 + dl:PAD + m0 + h0 + 512 + dl]
                            if _prod_engine('av', ch, j, kk) == 'pool':
                                nc.gpsimd.tensor_mul(t2, bch, vsl)
                            else:
                                nc.vector.tensor_mul(t2, bch, vsl)
                            for sub4 in range(4):
                                nc.tensor.matmul(
                                    o_gs[sub4][:, j * 128:(j + 1) * 128],
                                    t2[:, sub4 * 128:(sub4 + 1) * 128], identb,
                                    start=(kk == 0), stop=(kk == K2 - 1))
                        # evacuate this j's closed groups while the other j runs
                        for sub4 in range(4):
                            sub = half * 4 + sub4
                            dst = o_sb[:, sub, j * 128:(j + 1) * 128]
                            src = o_gs[sub4][:, j * 128:(j + 1) * 128]
                            nc.scalar.copy(out=dst, in_=src)
                    s0 = ch * SUBS + half * 4
                    nc.sync.dma_start(
                        out=out_view[:, s0:s0 + 4, :, :],
                        in_=o_sb[:, half * 4:half * 4 + 4, :].rearrange(
                            "p s (j c) -> p s j c", j=2))

            front(0)
            for ch in range(1, NCH):
                front(ch)
                back(ch - 1)
            back(NCH - 1)
    nc.compile()
    return nc


_NC_CACHE = None


def kernel(x: np.ndarray, W_qkv: np.ndarray) -> np.ndarray:
    global _NC_CACHE
    if _NC_CACHE is None:
        _NC_CACHE = build_nc()
    nc = _NC_CACHE

    x = np.ascontiguousarray(x, dtype=np.float32)
    W_qkv = np.ascontiguousarray(W_qkv, dtype=np.float32)
    ident, identb, ones_s, bkk, maskA, cntT = _host_consts()
    consts = {
        "w": W_qkv, "ident": ident, "identb": identb, "ones_s": ones_s,
        "bkk": bkk, "maskA": maskA, "cntT": cntT,
    }
    in_maps = [
        {"x": x[b].reshape(N, C).copy(), **consts} for b in range(B)
    ]
    res = run_bass_kernel_spmd(nc, in_maps, list(range(B)))
    out = np.stack([res.results[b]["out"].reshape(H, W, C) for b in range(B)])
    return out


if __name__ == "__main__":
    rng = np.random.default_rng(0)
    x = rng.standard_normal((B, H, W, C), dtype=np.float32)
    wq = (rng.standard_normal((3 * C, C), dtype=np.float32) * 0.02).astype(np.float32)
    out = kernel(x, wq)
    print("out", out.shape, out.dtype, float(np.abs(out).mean()))
